# revision 5
# baseline (speedup 1.0000x reference)
"""Trainium2 Bass kernel for a coverage-attention GRU decoder step.

Contract: kernel(**inputs) takes the full (unsharded) numpy inputs and
returns the full output, matching the reference:
    out (64, 144) f32, new_h (1, 64, 256) f32
Internally shards the batch (64) across 8 NeuronCores (8 per core),
replicating the (small) parameters.

Math note: the reference computes
    out_f = (alpha.sum(0) @ fc_w.T + fc_b).reshape(L, OF)
alpha_low/alpha_high are zero-filled by the problem spec, so the fc_w
term vanishes and out_f == fc_b.reshape(L, OF).  The host folds the
alpha-dependent term (exactly) only when alpha is nonzero; the device
kernel consumes out_f directly either way.

Layout notes (v2): all large operands are host-packed into SBUF-shaped
"mega" arrays — partition dim folded to 128 with zero padding, block
(j) major in the free dim — so each tensor is ONE contiguous DMA.
Activations are packed (j, b, l) so two batches are free-dim adjacent,
letting one N<=512 matmul compute a batch pair per weight load.
"""

import sys

try:
    import concourse  # noqa: F401
except ImportError:  # container stages the repo at /opt/trn_rl_repo
    sys.path.insert(0, "/opt/trn_rl_repo")

import numpy as np
import ml_dtypes

import concourse.bass as bass
import concourse.tile as tile
from concourse import bacc, mybir
from concourse.bass_utils import run_bass_kernel_spmd

F32 = mybir.dt.float32
BF16 = mybir.dt.bfloat16
NPBF = ml_dtypes.bfloat16

NCORES = 8
B = 64
BL = B // NCORES          # batches per core
EMB = 256
H = 256
NP_ = 512                 # attention projection dim
NCLS = 144
CL, CH = 684, 342
LL, LH = 256, 576
OF = 256
CTX = CL + CH

AX = mybir.AxisListType
AF = mybir.ActivationFunctionType
ALU = mybir.AluOpType


def _chunks(total, step=128):
    out = []
    s = 0
    while s < total:
        out.append((s, min(step, total - s)))
        s += step
    return out


C_LOW = _chunks(CL)       # 6 chunks (last 44 rows, zero-padded to 128)
C_HIGH = _chunks(CH)      # 3 chunks (last 86 rows)
# context rows: low chunks then high chunks (reference concatenates low,high)
CTX_CH = [(s, n) for (s, n) in C_LOW] + [(CL + s, n) for (s, n) in C_HIGH]
# high L split: pairs of these fit one PSUM bank (N = 2*ln <= 512)
HSUB = [(0, 256), (256, 512), (512, 576)]

_STATE = {}


# --------------------------------------------------------------------------
# device program
# --------------------------------------------------------------------------

def _build_program():
    nc = bacc.Bacc("TRN2", target_bir_lowering=False, debug=False,
                   enable_asserts=False, num_devices=NCORES)

    def din(name, shape, dt=F32):
        return nc.dram_tensor(name, shape, dt, kind="ExternalInput").ap()

    d = {}
    # per-core data (host-packed)
    d["sel_m"] = din("sel_m", [128, 2 * BL])             # one-hot, folded
    d["hT_m"] = din("hT_m", [128, 2 * BL], BF16)         # hidden.T folded
    d["h_b"] = din("h_b", [BL, H])
    d["A_low"] = din("A_low", [128, 6 * BL * LL], BF16)  # (j,b,l) packed
    d["A_high"] = din("A_high", [128, 3 * BL * LH], BF16)
    # replicated params (host-packed)
    d["emb_m"] = din("emb_m", [128, 2 * EMB])            # embedding folded f32
    d["eye8"] = din("eye8", [BL, BL])
    d["bsel"] = din("bsel", [BL, BL * 128], BF16)        # row-broadcast sel
    d["g1ih_m"] = din("g1ih_m", [128, 3 * 3 * H], BF16)  # [w.T;b] folded
    d["g1hh_m"] = din("g1hh_m", [128, 3 * 3 * H], BF16)
    d["g2ih_m"] = din("g2ih_m", [128, 10 * 3 * H], BF16)  # ctx-chunk blocks
    d["g2hh_m"] = din("g2hh_m", [128, 3 * 3 * H], BF16)
    d["upT_low"] = din("upT_low", [128, 2 * NP_], BF16)
    d["upT_high"] = din("upT_high", [128, 2 * NP_], BF16)
    d["wT_low"] = din("wT_low", [128, 8 * NP_], BF16)    # [U_a.T | U_f.T]
    d["wT_high"] = din("wT_high", [128, 5 * NP_], BF16)
    d["of_low"] = din("of_low", [128, 2 * 2 * LL], BF16)   # out_f.T dup'd
    d["of_high"] = din("of_high", [128, 2 * 2 * LH], BF16)
    d["nu8_lo"] = din("nu8_lo", [128, 4 * B], BF16)      # nu-masked, folded
    d["nu8_hi"] = din("nu8_hi", [128, 4 * B], BF16)
    d["WsT_m"] = din("WsT_m", [128, 2 * EMB], BF16)
    d["WcT_m"] = din("WcT_m", [128, 9 * EMB], BF16)
    d["WoT_m"] = din("WoT_m", [128, 2 * NCLS], BF16)

    out_d = nc.dram_tensor("out", [BL, NCLS], F32, kind="ExternalOutput").ap()
    newh_d = nc.dram_tensor("new_h", [BL, H], F32, kind="ExternalOutput").ap()

    from contextlib import ExitStack
    with tile.TileContext(nc) as tc, ExitStack() as ctx:
        P = ctx.enter_context(tc.tile_pool(name="persist", bufs=1))
        ST = ctx.enter_context(tc.tile_pool(name="stage", bufs=4))
        TH = ctx.enter_context(tc.tile_pool(name="tanhp", bufs=16))
        PSB = ctx.enter_context(tc.tile_pool(name="psbig", bufs=4, space="PSUM"))
        PSS = ctx.enter_context(tc.tile_pool(name="pssm", bufs=4, space="PSUM"))

        def ptile(shape, dt, tag):
            return P.tile(shape, dt, tag=tag, name=tag)

        def load(name, shape, dt):
            t = ptile(shape, dt, name)
            nc.sync.dma_start(t[:], d[name][:])
            return t

        # ---------------- persistent loads (one DMA each) ----------------
        A_low = load("A_low", [128, 6 * BL * LL], BF16)
        A_high = load("A_high", [128, 3 * BL * LH], BF16)
        wT = {"lo": load("wT_low", [128, 8 * NP_], BF16),
              "hi": load("wT_high", [128, 5 * NP_], BF16)}
        of = {"lo": load("of_low", [128, 4 * LL], BF16),
              "hi": load("of_high", [128, 4 * LH], BF16)}
        nu8 = {"lo": load("nu8_lo", [128, 4 * B], BF16),
               "hi": load("nu8_hi", [128, 4 * B], BF16)}
        upT = {"lo": load("upT_low", [128, 2 * NP_], BF16),
               "hi": load("upT_high", [128, 2 * NP_], BF16)}
        sel_m = load("sel_m", [128, 2 * BL], F32)
        hT_m = load("hT_m", [128, 2 * BL], BF16)
        h_b = load("h_b", [BL, H], F32)
        emb_m = load("emb_m", [128, 2 * EMB], F32)
        eye8 = load("eye8", [BL, BL], F32)
        bsel = load("bsel", [BL, BL * 128], BF16)
        g1ih_m = load("g1ih_m", [128, 9 * H], BF16)
        g1hh_m = load("g1hh_m", [128, 9 * H], BF16)
        g2ih_m = load("g2ih_m", [128, 30 * H], BF16)
        g2hh_m = load("g2hh_m", [128, 9 * H], BF16)
        WsT_m = load("WsT_m", [128, 2 * EMB], BF16)
        WcT_m = load("WcT_m", [128, 9 * EMB], BF16)
        WoT_m = load("WoT_m", [128, 2 * NCLS], BF16)

        ones8 = ptile([1, BL], BF16, "ones8")
        nc.vector.memset(ones8[:], 1.0)

        # ---------------- phase A: gather + gru1 ----------------
        # embedded.T = embedding.T @ onehot  (f32, exact)
        embT_f32, embT_bf = [], []
        for mi in range(2):
            ps = PSS.tile([128, BL], F32, tag="sm", name="sm")
            for j in range(2):
                nc.tensor.matmul(ps[:], emb_m[:, j * EMB + mi * 128:
                                              j * EMB + mi * 128 + 128],
                                 sel_m[:, j * BL:(j + 1) * BL],
                                 start=(j == 0), stop=(j == 1))
            ef = ptile([128, BL], F32, f"embTf{mi}")
            nc.vector.tensor_copy(ef[:], ps[:])
            eb = ptile([128, BL], BF16, f"embTb{mi}")
            nc.vector.tensor_copy(eb[:], ps[:])
            embT_f32.append(ef)
            embT_bf.append(eb)

        def gru(lh_ih, lh_hh, wih_m, whh_m, h_tile, tag):
            """lh_*: stationary (tile-ap, K) lists; last entry is the bias
            (ones8, 1).  wih_m/whh_m: folded moving weights whose j-th
            128-row block pairs with lh[j]; the bias row is the last
            block's row 0.  Returns (BL, H) f32 new hidden."""
            ps_r = PSS.tile([BL, H], F32, tag="sm", name="sm")
            ps_z = PSS.tile([BL, H], F32, tag="sm", name="sm")
            ps_in = PSS.tile([BL, H], F32, tag="sm", name="sm")
            ps_hn = PSS.tile([BL, H], F32, tag="sm", name="sm")

            def acc(ps, col0, sides):
                steps = []
                for lhs, mov in sides:
                    for j, (lt, lk) in enumerate(lhs):
                        steps.append((lt, lk, mov, j))
                for i, (lt, lk, mov, j) in enumerate(steps):
                    nc.tensor.matmul(
                        ps[:], lt[:lk, :],
                        mov[:lk, j * 3 * H + col0: j * 3 * H + col0 + H],
                        start=(i == 0), stop=(i == len(steps) - 1))

            acc(ps_r, 0, [(lh_ih, wih_m), (lh_hh, whh_m)])
            acc(ps_z, H, [(lh_ih, wih_m), (lh_hh, whh_m)])
            acc(ps_in, 2 * H, [(lh_ih, wih_m)])
            acc(ps_hn, 2 * H, [(lh_hh, whh_m)])

            r = ST.tile([BL, H], F32, tag=f"{tag}r", name=f"{tag}r", bufs=1)
            nc.scalar.activation(r[:], ps_r[:], AF.Sigmoid)
            z = ST.tile([BL, H], F32, tag=f"{tag}z", name=f"{tag}z", bufs=1)
            nc.scalar.activation(z[:], ps_z[:], AF.Sigmoid)
            rh = ST.tile([BL, H], F32, tag=f"{tag}rh", name=f"{tag}rh", bufs=1)
            nc.vector.tensor_tensor(rh[:], r[:], ps_hn[:], ALU.mult)
            npre = ST.tile([BL, H], F32, tag=f"{tag}np", name=f"{tag}np",
                           bufs=1)
            nc.vector.tensor_tensor(npre[:], rh[:], ps_in[:], ALU.add)
            n = ST.tile([BL, H], F32, tag=f"{tag}n", name=f"{tag}n", bufs=1)
            nc.scalar.activation(n[:], npre[:], AF.Tanh)
            dif = ST.tile([BL, H], F32, tag=f"{tag}d", name=f"{tag}d", bufs=1)
            nc.vector.tensor_tensor(dif[:], h_tile[:], n[:], ALU.subtract)
            zd = ST.tile([BL, H], F32, tag=f"{tag}zd", name=f"{tag}zd", bufs=1)
            nc.vector.tensor_tensor(zd[:], z[:], dif[:], ALU.mult)
            nh = ptile([BL, H], F32, f"{tag}_out")
            nc.vector.tensor_tensor(nh[:], n[:], zd[:], ALU.add)
            return nh

        eT = [(embT_bf[0], 128), (embT_bf[1], 128), (ones8, 1)]
        hT = [(hT_m[:, 0:BL], 128), (hT_m[:, BL:2 * BL], 128), (ones8, 1)]
        pred = gru(eT, hT, g1ih_m, g1hh_m, h_b, "g1")

        # pred.T in bf16 (2 chunks of (128, BL))
        predT = []
        for ki in range(2):
            ps = PSS.tile([128, BL], F32, tag="sm", name="sm")
            nc.tensor.transpose(ps[:], pred[:, ki * 128:(ki + 1) * 128],
                                eye8[:])
            t = ptile([128, BL], BF16, f"predT{ki}")
            nc.vector.tensor_copy(t[:], ps[:])
            predT.append(t)

        # u_pred.T per block: 4 chunks of (128, BL) f32 (ACT bias columns)
        up_sb = {}
        for blk in ("lo", "hi"):
            res = []
            for mi in range(4):
                ps = PSS.tile([128, BL], F32, tag="sm", name="sm")
                for j in range(2):
                    nc.tensor.matmul(
                        ps[:], upT[blk][:, j * NP_ + mi * 128:
                                        j * NP_ + mi * 128 + 128],
                        predT[j][:], start=(j == 0), stop=(j == 1))
                t = ptile([128, BL], F32, f"up{blk}{mi}")
                nc.vector.tensor_copy(t[:], ps[:])
                res.append(t)
            up_sb[blk] = res

        # ---------------- phase B: coverage attention ----------------
        def block(blk, nC, L, Lsubs, A_t, nK):
            """nK = nC a-blocks + 2 out_f blocks.  Returns ctxT_bf chunks."""
            Av = A_t.rearrange("p (x l) -> p x l", l=L)   # (128, nC*BL, L)
            pe = [PSS.tile([BL, l1 - l0], F32, tag="sm", name="sm")
                  for (l0, l1) in Lsubs]
            for mi in range(4):
                ths = [TH.tile([128, L], BF16, tag="tanh", name="tanh")
                       for _ in range(BL)]
                for si, (l0, l1) in enumerate(Lsubs):
                    ln = l1 - l0
                    pts = [PSB.tile([128, 512], F32, tag="pt", name="pt")
                           for _ in range(4)]
                    for kc in range(nK):
                        w = wT[blk][:, kc * NP_ + mi * 128:
                                    kc * NP_ + mi * 128 + 128]
                        for q in range(4):
                            if kc < nC:
                                rhs = Av[:, kc * BL + 2 * q:
                                         kc * BL + 2 * q + 2, l0:l1]
                            else:
                                base = (kc - nC) * 2 * L + 2 * l0
                                rhs = of[blk][:, base: base + 2 * ln]
                            nc.tensor.matmul(pts[q][:, 0:2 * ln], w, rhs,
                                             start=(kc == 0),
                                             stop=(kc == nK - 1))
                    for b in range(BL):
                        nc.scalar.activation(
                            ths[b][:, l0:l1],
                            pts[b // 2][:, (b % 2) * ln:(b % 2 + 1) * ln],
                            AF.Tanh, bias=up_sb[blk][mi][:, b:b + 1])
                for b in range(BL):
                    for si, (l0, l1) in enumerate(Lsubs):
                        nc.tensor.matmul(
                            pe[si][:, :],
                            nu8[blk][:, mi * B + b * BL: mi * B + (b + 1) * BL],
                            ths[b][:, l0:l1],
                            start=(mi == 0 and b == 0),
                            stop=(mi == 3 and b == BL - 1))

            # softmax over L (rows = batches)
            et = ST.tile([BL, L], F32, tag="et", name="et", bufs=2)
            for si, (l0, l1) in enumerate(Lsubs):
                nc.vector.tensor_copy(et[:, l0:l1], pe[si][:, :])
            nmax = ST.tile([BL, 1], F32, tag="nmax", name="nmax")
            nc.vector.tensor_reduce(nmax[:], et[:], axis=AX.X, op=ALU.max,
                                    negate=True)
            expt = ST.tile([BL, L], BF16, tag="expt", name="expt", bufs=2)
            sums = ST.tile([BL, 1], F32, tag="sums", name="sums")
            nc.scalar.activation(expt[:], et[:], AF.Exp, bias=nmax[:],
                                 accum_out=sums[:])
            recip = ST.tile([BL, 1], F32, tag="recip", name="recip")
            nc.vector.reciprocal(recip[:], sums[:])
            alph = ST.tile([BL, L], BF16, tag="alph", name="alph", bufs=2)
            nc.vector.tensor_scalar_mul(alph[:], expt[:], recip[:])

            # ctx.T chunks (c on partitions, batch on free); padded rows
            # of A are zero so they accumulate exact zeros.
            ctx_f = [ptile([128, BL], F32, f"{blk}ctx{ci}") for ci in range(nC)]
            for b in range(BL):
                ab = ST.tile([128, L], BF16, tag="ab", name="ab")
                for si, (l0, l1) in enumerate(Lsubs):
                    ln = l1 - l0
                    abp = PSS.tile([128, 512], F32, tag="sm", name="sm")
                    nc.tensor.matmul(abp[:, :ln],
                                     bsel[:, b * 128:(b + 1) * 128],
                                     alph[:, l0:l1], start=True, stop=True)
                    nc.vector.tensor_copy(ab[:, l0:l1], abp[:, :ln])
                for ci in range(nC):
                    junk = ST.tile([128, L], BF16, tag="junk", name="junk",
                                   bufs=2)
                    nc.vector.scalar_tensor_tensor(
                        junk[:], Av[:, ci * BL + b, :], 1.0, ab[:],
                        ALU.mult, ALU.mult,
                        accum_out=ctx_f[ci][:, b:b + 1])
            res = []
            for ci in range(nC):
                t = ptile([128, BL], BF16, f"{blk}ctxb{ci}")
                nc.vector.tensor_copy(t[:], ctx_f[ci][:])
                res.append(t)
            return res

        ctx_low = block("lo", 6, LL, [(0, LL)], A_low, 8)
        ctx_high = block("hi", 3, LH, HSUB, A_high, 5)
        ctxT = ctx_low + ctx_high            # 9 chunks, zero-padded rows

        # ---------------- phase C: gru2 + output ----------------
        lh_ih2 = [(t, 128) for t in ctxT] + [(ones8, 1)]
        lh_hh2 = [(predT[0], 128), (predT[1], 128), (ones8, 1)]
        new_h = gru(lh_ih2, lh_hh2, g2ih_m, g2hh_m, pred, "g2")
        nc.sync.dma_start(newh_d[:], new_h[:])

        # new_h.T bf16
        nhT = []
        for ki in range(2):
            ps = PSS.tile([128, BL], F32, tag="sm", name="sm")
            nc.tensor.transpose(ps[:], new_h[:, ki * 128:(ki + 1) * 128],
                                eye8[:])
            t = ptile([128, BL], BF16, f"nhT{ki}")
            nc.vector.tensor_copy(t[:], ps[:])
            nhT.append(t)

        # S.T = embedded.T + W_s @ new_h.T + W_c @ ctx.T   (per EMB chunk)
        sT = []
        for mi in range(2):
            ps = PSS.tile([128, BL], F32, tag="sm", name="sm")
            steps = [(WsT_m[:, j * EMB + mi * 128: j * EMB + mi * 128 + 128],
                      nhT[j]) for j in range(2)]
            steps += [(WcT_m[:, j * EMB + mi * 128: j * EMB + mi * 128 + 128],
                       ctxT[j]) for j in range(9)]
            for i, (w, r) in enumerate(steps):
                nc.tensor.matmul(ps[:], w, r[:], start=(i == 0),
                                 stop=(i == len(steps) - 1))
            t = ptile([128, BL], BF16, f"sT{mi}")
            nc.vector.tensor_tensor(t[:], ps[:], embT_f32[mi][:], ALU.add)
            sT.append(t)

        # out = S @ W_o.T : (BL, NCLS)
        ps = PSS.tile([BL, NCLS], F32, tag="sm", name="sm")
        for ki in range(2):
            nc.tensor.matmul(ps[:], sT[ki][:],
                             WoT_m[:, ki * NCLS:(ki + 1) * NCLS],
                             start=(ki == 0), stop=(ki == 1))
        out_sb = ST.tile([BL, NCLS], F32, tag="outsb", name="outsb")
        nc.vector.tensor_copy(out_sb[:], ps[:])
        nc.sync.dma_start(out_d[:], out_sb[:])

    nc.compile()
    return nc


# --------------------------------------------------------------------------
# host side
# --------------------------------------------------------------------------

def _fold128(M, dt):
    """(R, W) -> (128, ceil(R/128)*W): 128-row blocks along the free dim,
    zero padded."""
    R, W = M.shape
    nj = (R + 127) // 128
    out = np.zeros((128, nj, W), dt)
    for j in range(nj):
        kk = min(128, R - j * 128)
        out[:kk, j] = M[j * 128: j * 128 + kk]
    return out.reshape(128, nj * W)


def _fold_blocks(M, bounds, dt):
    """rows of M grouped by (start, size) bounds, each padded to 128."""
    W = M.shape[1]
    out = np.zeros((128, len(bounds), W), dt)
    for j, (s, n) in enumerate(bounds):
        out[:n, j] = M[s:s + n]
    return out.reshape(128, len(bounds) * W)


def _bsel():
    s = np.zeros((BL, BL * 128), np.float32)
    for b in range(BL):
        s[b, b * 128:(b + 1) * 128] = 1.0
    return s.astype(NPBF)


def _nu_mask(nu):
    D = np.zeros((NP_, BL, BL), np.float32)
    D[:, np.arange(BL), np.arange(BL)] = np.asarray(nu, np.float32)[:, None]
    return _fold128(D.reshape(NP_, B), NPBF)


def _pack_a(a, nC, L):
    """(BL, C, L) f32 -> (128, nC*BL*L) bf16, (j, b, l) free layout,
    zero-padded partial chunk."""
    C = a.shape[1]
    out = np.zeros((128, nC, BL, L), NPBF)
    for j in range(nC):
        kk = min(128, C - j * 128)
        out[:kk, j] = np.moveaxis(a[:, j * 128: j * 128 + kk], 0, 1)
    return out.reshape(128, nC * BL * L)


def _dup_outf(outfT, Lsubs):
    """(OF, L) out_f.T -> (128, 2*sum(2*ln)): each (o, sub) chunk
    duplicated side by side for batch-pair matmuls."""
    parts = []
    for o in range(2):
        chunk = outfT[o * 128:(o + 1) * 128]
        for (l0, l1) in Lsubs:
            c = chunk[:, l0:l1]
            parts.append(np.concatenate([c, c], axis=1))
    return np.ascontiguousarray(np.concatenate(parts, axis=1)).astype(NPBF)


def _prep_in_maps(inputs):
    f32 = np.float32

    def A(name):
        return np.asarray(inputs[name])

    x = A("x").reshape(B).astype(np.int64)
    hidden = A("hidden").astype(f32)          # (1, B, H)
    low = A("low_res").astype(f32).reshape(B, CL, LL)
    high = A("high_res").astype(f32).reshape(B, CH, LH)

    def outf(alpha, fc_w, fc_b, L):
        a = np.asarray(alpha, f32)
        o = np.asarray(fc_b, f32).reshape(L, OF)
        if np.any(a):
            o = o + (a.sum(0) @ np.asarray(fc_w, f32).T).reshape(L, OF)
        return np.ascontiguousarray(o.T)          # (OF, L) f32

    T = lambda name: np.asarray(inputs[name], f32).T

    def gru_w(w_name, b_name, bounds):
        wt = T(w_name)
        bias = A(b_name).astype(f32)[None]
        M = np.vstack([wt, bias])
        return _fold_blocks(M, bounds + [(M.shape[0] - 1, 1)], NPBF)

    rep = {
        "emb_m": _fold128(A("embedding").astype(f32), f32),
        "eye8": np.eye(BL, dtype=f32),
        "bsel": _bsel(),
        "g1ih_m": gru_w("gru1_w_ih", "gru1_b_ih", _chunks(EMB)),
        "g1hh_m": gru_w("gru1_w_hh", "gru1_b_hh", _chunks(H)),
        "g2ih_m": gru_w("gru2_w_ih", "gru2_b_ih", CTX_CH),
        "g2hh_m": gru_w("gru2_w_hh", "gru2_b_hh", _chunks(H)),
        "upT_low": _fold128(T("U_pred_low"), NPBF),
        "upT_high": _fold128(T("U_pred_high"), NPBF),
        "wT_low": np.concatenate([_fold128(T("U_a_low"), NPBF),
                                  _fold128(T("U_f_low"), NPBF)], axis=1),
        "wT_high": np.concatenate([_fold128(T("U_a_high"), NPBF),
                                   _fold128(T("U_f_high"), NPBF)], axis=1),
        "of_low": _dup_outf(outf(inputs["alpha_low"], inputs["fc_w_low"],
                                 inputs["fc_b_low"], LL), [(0, LL)]),
        "of_high": _dup_outf(outf(inputs["alpha_high"], inputs["fc_w_high"],
                                  inputs["fc_b_high"], LH), HSUB),
        "nu8_lo": _nu_mask(A("nu_low")),
        "nu8_hi": _nu_mask(A("nu_high")),
        "WsT_m": _fold128(T("W_s"), NPBF),
        "WcT_m": _fold_blocks(T("W_c"), CTX_CH, NPBF),
        "WoT_m": _fold128(T("W_o"), NPBF),
    }

    in_maps = []
    for c in range(NCORES):
        b0, b1 = c * BL, (c + 1) * BL
        sel = np.zeros((NCLS, BL), f32)
        sel[x[b0:b1], np.arange(BL)] = 1.0
        m = dict(rep)
        m["sel_m"] = _fold128(sel, f32)
        m["hT_m"] = _fold128(np.ascontiguousarray(hidden[0, b0:b1].T), NPBF)
        m["h_b"] = np.ascontiguousarray(hidden[0, b0:b1])
        m["A_low"] = _pack_a(low[b0:b1], 6, LL)
        m["A_high"] = _pack_a(high[b0:b1], 3, LH)
        in_maps.append(m)
    return in_maps


def _run(inputs, trace=False, trace_cores=None):
    if "nc" not in _STATE:
        _STATE["nc"] = _build_program()
    nc = _STATE["nc"]
    in_maps = _prep_in_maps(inputs)
    res = run_bass_kernel_spmd(nc, in_maps, list(range(NCORES)), trace=trace,
                               trace_cores=trace_cores)
    out = np.concatenate([res.results[c]["out"] for c in range(NCORES)], axis=0)
    new_h = np.concatenate([res.results[c]["new_h"] for c in range(NCORES)],
                           axis=0)[None]
    return (out.astype(np.float32), new_h.astype(np.float32)), res


def kernel(**inputs):
    (out, new_h), _ = _run(inputs)
    return out, new_h


# revision 6
# speedup vs baseline: 1.0623x; 1.0623x over previous
"""Trainium2 Bass kernel for a coverage-attention GRU decoder step.

Contract: kernel(**inputs) takes the full (unsharded) numpy inputs and
returns the full output, matching the reference:
    out (64, 144) f32, new_h (1, 64, 256) f32
Internally shards the batch (64) across 8 NeuronCores (8 per core),
replicating the (small) parameters.

Math note: the reference computes
    out_f = (alpha.sum(0) @ fc_w.T + fc_b).reshape(L, OF)
alpha_low/alpha_high are zero-filled by the problem spec, so the fc_w
term vanishes and out_f == fc_b.reshape(L, OF).  The host folds the
alpha-dependent term (exactly) only when alpha is nonzero; the device
kernel consumes out_f directly either way.

Layout notes (v2): all large operands are host-packed into SBUF-shaped
"mega" arrays — partition dim folded to 128 with zero padding, block
(j) major in the free dim — so each tensor is ONE contiguous DMA.
Activations are packed (j, b, l) so two batches are free-dim adjacent,
letting one N<=512 matmul compute a batch pair per weight load.
"""

import sys

try:
    import concourse  # noqa: F401
except ImportError:  # container stages the repo at /opt/trn_rl_repo
    sys.path.insert(0, "/opt/trn_rl_repo")

import numpy as np
import ml_dtypes

import concourse.bass as bass
import concourse.tile as tile
from concourse import bacc, mybir
from concourse.bass_utils import run_bass_kernel_spmd

F32 = mybir.dt.float32
BF16 = mybir.dt.bfloat16
NPBF = ml_dtypes.bfloat16

NCORES = 8
B = 64
BL = B // NCORES          # batches per core
EMB = 256
H = 256
NP_ = 512                 # attention projection dim
NCLS = 144
CL, CH = 684, 342
LL, LH = 256, 576
OF = 256
CTX = CL + CH

AX = mybir.AxisListType
AF = mybir.ActivationFunctionType
ALU = mybir.AluOpType


def _chunks(total, step=128):
    out = []
    s = 0
    while s < total:
        out.append((s, min(step, total - s)))
        s += step
    return out


C_LOW = _chunks(CL)       # 6 chunks (last 44 rows, zero-padded to 128)
C_HIGH = _chunks(CH)      # 3 chunks (last 86 rows)
# context rows: low chunks then high chunks (reference concatenates low,high)
CTX_CH = [(s, n) for (s, n) in C_LOW] + [(CL + s, n) for (s, n) in C_HIGH]
# high L split: pairs of these fit one PSUM bank (N = 2*ln <= 512)
HSUB = [(0, 256), (256, 512), (512, 576)]

_STATE = {}


# --------------------------------------------------------------------------
# device program
# --------------------------------------------------------------------------

def _build_program():
    nc = bacc.Bacc("TRN2", target_bir_lowering=False, debug=False,
                   enable_asserts=False, num_devices=NCORES)

    def din(name, shape, dt=F32):
        return nc.dram_tensor(name, shape, dt, kind="ExternalInput").ap()

    d = {}
    # per-core data (host-packed)
    d["sel_m"] = din("sel_m", [128, 2 * BL])             # one-hot, folded
    d["hT_m"] = din("hT_m", [128, 2 * BL], BF16)         # hidden.T folded
    d["h_b"] = din("h_b", [BL, H])
    d["A_low"] = din("A_low", [128, 6 * BL * LL], BF16)  # (j,b,l) packed
    d["A_high"] = din("A_high", [128, 3 * BL * LH], BF16)
    # replicated params (host-packed)
    d["emb_m"] = din("emb_m", [128, 2 * EMB])            # embedding folded f32
    d["eye8"] = din("eye8", [BL, BL])
    d["bsel"] = din("bsel", [BL, BL * 128], BF16)        # row-broadcast sel
    d["g1ih_m"] = din("g1ih_m", [128, 3 * 3 * H], BF16)  # [w.T;b] folded
    d["g1hh_m"] = din("g1hh_m", [128, 3 * 3 * H], BF16)
    d["g2ih_m"] = din("g2ih_m", [128, 10 * 3 * H], BF16)  # ctx-chunk blocks
    d["g2hh_m"] = din("g2hh_m", [128, 3 * 3 * H], BF16)
    d["upT_low"] = din("upT_low", [128, 2 * NP_], BF16)
    d["upT_high"] = din("upT_high", [128, 2 * NP_], BF16)
    d["wT_low"] = din("wT_low", [128, 8 * NP_], BF16)    # [U_a.T | U_f.T]
    d["wT_high"] = din("wT_high", [128, 5 * NP_], BF16)
    d["of_low"] = din("of_low", [128, 2 * 2 * LL], BF16)   # out_f.T dup'd
    d["of_high"] = din("of_high", [128, 2 * 2 * LH], BF16)
    d["nu8_lo"] = din("nu8_lo", [128, 4 * B], BF16)      # nu-masked, folded
    d["nu8_hi"] = din("nu8_hi", [128, 4 * B], BF16)
    d["WsT_m"] = din("WsT_m", [128, 2 * EMB], BF16)
    d["WcT_m"] = din("WcT_m", [128, 9 * EMB], BF16)
    d["WoT_m"] = din("WoT_m", [128, 2 * NCLS], BF16)

    out_d = nc.dram_tensor("out", [BL, NCLS], F32, kind="ExternalOutput").ap()
    newh_d = nc.dram_tensor("new_h", [BL, H], F32, kind="ExternalOutput").ap()

    from contextlib import ExitStack
    with tile.TileContext(nc) as tc, ExitStack() as ctx:
        P = ctx.enter_context(tc.tile_pool(name="persist", bufs=1))
        ST = ctx.enter_context(tc.tile_pool(name="stage", bufs=4))
        TH = ctx.enter_context(tc.tile_pool(name="tanhp", bufs=16))
        PSB = ctx.enter_context(tc.tile_pool(name="psbig", bufs=4, space="PSUM"))
        PSS = ctx.enter_context(tc.tile_pool(name="pssm", bufs=4, space="PSUM"))

        def ptile(shape, dt, tag):
            return P.tile(shape, dt, tag=tag, name=tag)

        def load(name, shape, dt):
            t = ptile(shape, dt, name)
            nc.sync.dma_start(t[:], d[name][:])
            return t

        # ------------- persistent loads (small first; A last, chunked) -----
        sel_m = load("sel_m", [128, 2 * BL], F32)
        emb_m = load("emb_m", [128, 2 * EMB], F32)
        hT_m = load("hT_m", [128, 2 * BL], BF16)
        h_b = load("h_b", [BL, H], F32)
        eye8 = load("eye8", [BL, BL], F32)
        bsel = load("bsel", [BL, BL * 128], BF16)
        g1ih_m = load("g1ih_m", [128, 9 * H], BF16)
        g1hh_m = load("g1hh_m", [128, 9 * H], BF16)
        upT = {"lo": load("upT_low", [128, 2 * NP_], BF16),
               "hi": load("upT_high", [128, 2 * NP_], BF16)}
        nu8 = {"lo": load("nu8_lo", [128, 4 * B], BF16),
               "hi": load("nu8_hi", [128, 4 * B], BF16)}
        wT = {"lo": load("wT_low", [128, 8 * NP_], BF16),
              "hi": load("wT_high", [128, 5 * NP_], BF16)}
        of = {"lo": load("of_low", [128, 4 * LL], BF16),
              "hi": load("of_high", [128, 4 * LH], BF16)}
        g2ih_m = load("g2ih_m", [128, 30 * H], BF16)
        g2hh_m = load("g2hh_m", [128, 9 * H], BF16)
        WsT_m = load("WsT_m", [128, 2 * EMB], BF16)
        WcT_m = load("WcT_m", [128, 9 * EMB], BF16)
        WoT_m = load("WoT_m", [128, 2 * NCLS], BF16)
        # A tiles: one DMA per k-chunk so matmuls start on chunk arrival
        A_low = ptile([128, 6 * BL * LL], BF16, "A_low")
        for j in range(6):
            nc.sync.dma_start(A_low[:, j * BL * LL:(j + 1) * BL * LL],
                              d["A_low"][:, j * BL * LL:(j + 1) * BL * LL])
        A_high = ptile([128, 3 * BL * LH], BF16, "A_high")
        for j in range(3):
            nc.scalar.dma_start(A_high[:, j * BL * LH:(j + 1) * BL * LH],
                                d["A_high"][:, j * BL * LH:(j + 1) * BL * LH])

        ones8 = ptile([1, BL], BF16, "ones8")
        nc.vector.memset(ones8[:], 1.0)

        # ---------------- phase A: gather + gru1 ----------------
        # embedded.T = embedding.T @ onehot  (f32, exact)
        embT_f32, embT_bf = [], []
        for mi in range(2):
            ps = PSS.tile([128, BL], F32, tag="sm", name="sm")
            for j in range(2):
                nc.tensor.matmul(ps[:], emb_m[:, j * EMB + mi * 128:
                                              j * EMB + mi * 128 + 128],
                                 sel_m[:, j * BL:(j + 1) * BL],
                                 start=(j == 0), stop=(j == 1))
            ef = ptile([128, BL], F32, f"embTf{mi}")
            nc.vector.tensor_copy(ef[:], ps[:])
            eb = ptile([128, BL], BF16, f"embTb{mi}")
            nc.vector.tensor_copy(eb[:], ps[:])
            embT_f32.append(ef)
            embT_bf.append(eb)

        def gru(lh_ih, lh_hh, wih_m, whh_m, h_tile, tag):
            """lh_*: stationary (tile-ap, K) lists; last entry is the bias
            (ones8, 1).  wih_m/whh_m: folded moving weights whose j-th
            128-row block pairs with lh[j]; the bias row is the last
            block's row 0.  Returns (BL, H) f32 new hidden."""
            ps_r = PSS.tile([BL, H], F32, tag="sm", name="sm")
            ps_z = PSS.tile([BL, H], F32, tag="sm", name="sm")
            ps_in = PSS.tile([BL, H], F32, tag="sm", name="sm")
            ps_hn = PSS.tile([BL, H], F32, tag="sm", name="sm")

            def acc(ps, col0, sides):
                steps = []
                for lhs, mov in sides:
                    for j, (lt, lk) in enumerate(lhs):
                        steps.append((lt, lk, mov, j))
                for i, (lt, lk, mov, j) in enumerate(steps):
                    nc.tensor.matmul(
                        ps[:], lt[:lk, :],
                        mov[:lk, j * 3 * H + col0: j * 3 * H + col0 + H],
                        start=(i == 0), stop=(i == len(steps) - 1))

            acc(ps_r, 0, [(lh_ih, wih_m), (lh_hh, whh_m)])
            acc(ps_z, H, [(lh_ih, wih_m), (lh_hh, whh_m)])
            acc(ps_in, 2 * H, [(lh_ih, wih_m)])
            acc(ps_hn, 2 * H, [(lh_hh, whh_m)])

            r = ST.tile([BL, H], F32, tag=f"{tag}r", name=f"{tag}r", bufs=1)
            nc.scalar.activation(r[:], ps_r[:], AF.Sigmoid)
            z = ST.tile([BL, H], F32, tag=f"{tag}z", name=f"{tag}z", bufs=1)
            nc.scalar.activation(z[:], ps_z[:], AF.Sigmoid)
            rh = ST.tile([BL, H], F32, tag=f"{tag}rh", name=f"{tag}rh", bufs=1)
            nc.vector.tensor_tensor(rh[:], r[:], ps_hn[:], ALU.mult)
            npre = ST.tile([BL, H], F32, tag=f"{tag}np", name=f"{tag}np",
                           bufs=1)
            nc.vector.tensor_tensor(npre[:], rh[:], ps_in[:], ALU.add)
            n = ST.tile([BL, H], F32, tag=f"{tag}n", name=f"{tag}n", bufs=1)
            nc.scalar.activation(n[:], npre[:], AF.Tanh)
            dif = ST.tile([BL, H], F32, tag=f"{tag}d", name=f"{tag}d", bufs=1)
            nc.vector.tensor_tensor(dif[:], h_tile[:], n[:], ALU.subtract)
            zd = ST.tile([BL, H], F32, tag=f"{tag}zd", name=f"{tag}zd", bufs=1)
            nc.vector.tensor_tensor(zd[:], z[:], dif[:], ALU.mult)
            nh = ptile([BL, H], F32, f"{tag}_out")
            nc.vector.tensor_tensor(nh[:], n[:], zd[:], ALU.add)
            return nh

        eT = [(embT_bf[0], 128), (embT_bf[1], 128), (ones8, 1)]
        hT = [(hT_m[:, 0:BL], 128), (hT_m[:, BL:2 * BL], 128), (ones8, 1)]
        pred = gru(eT, hT, g1ih_m, g1hh_m, h_b, "g1")

        # pred.T in bf16 (2 chunks of (128, BL))
        predT = []
        for ki in range(2):
            ps = PSS.tile([128, BL], F32, tag="sm", name="sm")
            nc.tensor.transpose(ps[:], pred[:, ki * 128:(ki + 1) * 128],
                                eye8[:])
            t = ptile([128, BL], BF16, f"predT{ki}")
            nc.vector.tensor_copy(t[:], ps[:])
            predT.append(t)

        # u_pred.T per block: 4 chunks of (128, BL) f32 (ACT bias columns)
        up_sb = {}
        for blk in ("lo", "hi"):
            res = []
            for mi in range(4):
                ps = PSS.tile([128, BL], F32, tag="sm", name="sm")
                for j in range(2):
                    nc.tensor.matmul(
                        ps[:], upT[blk][:, j * NP_ + mi * 128:
                                        j * NP_ + mi * 128 + 128],
                        predT[j][:], start=(j == 0), stop=(j == 1))
                t = ptile([128, BL], F32, f"up{blk}{mi}")
                nc.vector.tensor_copy(t[:], ps[:])
                res.append(t)
            up_sb[blk] = res

        # ---------------- phase B: coverage attention ----------------
        def block(blk, nC, L, Lsubs, A_t, nK):
            """nK = nC a-blocks + 2 out_f blocks.  Returns ctxT_bf chunks."""
            Av = A_t.rearrange("p (x l) -> p x l", l=L)   # (128, nC*BL, L)
            pe = [PSS.tile([BL, l1 - l0], F32, tag="sm", name="sm")
                  for (l0, l1) in Lsubs]
            for mi in range(4):
                ths = [TH.tile([128, L], BF16, tag="tanh", name="tanh")
                       for _ in range(BL)]
                for si, (l0, l1) in enumerate(Lsubs):
                    ln = l1 - l0
                    pts = [PSB.tile([128, 512], F32, tag="pt", name="pt")
                           for _ in range(4)]
                    for kc in range(nK):
                        w = wT[blk][:, kc * NP_ + mi * 128:
                                    kc * NP_ + mi * 128 + 128]
                        for q in range(4):
                            if kc < nC:
                                rhs = Av[:, kc * BL + 2 * q:
                                         kc * BL + 2 * q + 2, l0:l1]
                            else:
                                base = (kc - nC) * 2 * L + 2 * l0
                                rhs = of[blk][:, base: base + 2 * ln]
                            nc.tensor.matmul(pts[q][:, 0:2 * ln], w, rhs,
                                             start=(kc == 0),
                                             stop=(kc == nK - 1))
                    for b in range(BL):
                        nc.scalar.activation(
                            ths[b][:, l0:l1],
                            pts[b // 2][:, (b % 2) * ln:(b % 2 + 1) * ln],
                            AF.Tanh, bias=up_sb[blk][mi][:, b:b + 1])
                for b in range(BL):
                    for si, (l0, l1) in enumerate(Lsubs):
                        nc.tensor.matmul(
                            pe[si][:, :],
                            nu8[blk][:, mi * B + b * BL: mi * B + (b + 1) * BL],
                            ths[b][:, l0:l1],
                            start=(mi == 0 and b == 0),
                            stop=(mi == 3 and b == BL - 1))

            # softmax over L (rows = batches)
            et = ST.tile([BL, L], F32, tag="et", name="et", bufs=2)
            for si, (l0, l1) in enumerate(Lsubs):
                nc.vector.tensor_copy(et[:, l0:l1], pe[si][:, :])
            nmax = ST.tile([BL, 1], F32, tag="nmax", name="nmax")
            nc.vector.tensor_reduce(nmax[:], et[:], axis=AX.X, op=ALU.max,
                                    negate=True)
            expt = ST.tile([BL, L], BF16, tag="expt", name="expt", bufs=2)
            sums = ST.tile([BL, 1], F32, tag="sums", name="sums")
            nc.scalar.activation(expt[:], et[:], AF.Exp, bias=nmax[:],
                                 accum_out=sums[:])
            recip = ST.tile([BL, 1], F32, tag="recip", name="recip")
            nc.vector.reciprocal(recip[:], sums[:])
            alph = ST.tile([BL, L], BF16, tag="alph", name="alph", bufs=2)
            nc.vector.tensor_scalar_mul(alph[:], expt[:], recip[:])

            # ctx.T chunks (c on partitions, batch on free); padded rows
            # of A are zero so they accumulate exact zeros.
            abs_ = []
            for b in range(BL):
                ab = ST.tile([128, L], BF16, tag="ab", name="ab", bufs=8)
                for si, (l0, l1) in enumerate(Lsubs):
                    ln = l1 - l0
                    abp = PSS.tile([128, 512], F32, tag="sm", name="sm")
                    nc.tensor.matmul(abp[:, :ln],
                                     bsel[:, b * 128:(b + 1) * 128],
                                     alph[:, l0:l1], start=True, stop=True)
                    nc.scalar.copy(ab[:, l0:l1], abp[:, :ln])
                abs_.append(ab)
            res = []
            for ci in range(nC):
                ctx_f = ptile([128, BL], F32, f"{blk}ctx{ci}")
                for b in range(BL):
                    junk = ST.tile([128, L], BF16, tag="junk", name="junk",
                                   bufs=2)
                    nc.vector.scalar_tensor_tensor(
                        junk[:], Av[:, ci * BL + b, :], 1.0, abs_[b][:],
                        ALU.mult, ALU.mult,
                        accum_out=ctx_f[:, b:b + 1])
                t = ptile([128, BL], BF16, f"{blk}ctxb{ci}")
                nc.vector.tensor_copy(t[:], ctx_f[:])
                res.append(t)
            return res

        ctx_low = block("lo", 6, LL, [(0, LL)], A_low, 8)
        ctx_high = block("hi", 3, LH, HSUB, A_high, 5)
        ctxT = ctx_low + ctx_high            # 9 chunks, zero-padded rows

        # ---------------- phase C: gru2 + output ----------------
        lh_ih2 = [(t, 128) for t in ctxT] + [(ones8, 1)]
        lh_hh2 = [(predT[0], 128), (predT[1], 128), (ones8, 1)]
        new_h = gru(lh_ih2, lh_hh2, g2ih_m, g2hh_m, pred, "g2")
        nc.sync.dma_start(newh_d[:], new_h[:])

        # new_h.T bf16
        nhT = []
        for ki in range(2):
            ps = PSS.tile([128, BL], F32, tag="sm", name="sm")
            nc.tensor.transpose(ps[:], new_h[:, ki * 128:(ki + 1) * 128],
                                eye8[:])
            t = ptile([128, BL], BF16, f"nhT{ki}")
            nc.vector.tensor_copy(t[:], ps[:])
            nhT.append(t)

        # S.T = embedded.T + W_s @ new_h.T + W_c @ ctx.T   (per EMB chunk)
        sT = []
        for mi in range(2):
            ps = PSS.tile([128, BL], F32, tag="sm", name="sm")
            steps = [(WsT_m[:, j * EMB + mi * 128: j * EMB + mi * 128 + 128],
                      nhT[j]) for j in range(2)]
            steps += [(WcT_m[:, j * EMB + mi * 128: j * EMB + mi * 128 + 128],
                       ctxT[j]) for j in range(9)]
            for i, (w, r) in enumerate(steps):
                nc.tensor.matmul(ps[:], w, r[:], start=(i == 0),
                                 stop=(i == len(steps) - 1))
            t = ptile([128, BL], BF16, f"sT{mi}")
            nc.vector.tensor_tensor(t[:], ps[:], embT_f32[mi][:], ALU.add)
            sT.append(t)

        # out = S @ W_o.T : (BL, NCLS)
        ps = PSS.tile([BL, NCLS], F32, tag="sm", name="sm")
        for ki in range(2):
            nc.tensor.matmul(ps[:], sT[ki][:],
                             WoT_m[:, ki * NCLS:(ki + 1) * NCLS],
                             start=(ki == 0), stop=(ki == 1))
        out_sb = ST.tile([BL, NCLS], F32, tag="outsb", name="outsb")
        nc.vector.tensor_copy(out_sb[:], ps[:])
        nc.sync.dma_start(out_d[:], out_sb[:])

    nc.compile()
    return nc


# --------------------------------------------------------------------------
# host side
# --------------------------------------------------------------------------

def _fold128(M, dt):
    """(R, W) -> (128, ceil(R/128)*W): 128-row blocks along the free dim,
    zero padded."""
    R, W = M.shape
    nj = (R + 127) // 128
    out = np.zeros((128, nj, W), dt)
    for j in range(nj):
        kk = min(128, R - j * 128)
        out[:kk, j] = M[j * 128: j * 128 + kk]
    return out.reshape(128, nj * W)


def _fold_blocks(M, bounds, dt):
    """rows of M grouped by (start, size) bounds, each padded to 128."""
    W = M.shape[1]
    out = np.zeros((128, len(bounds), W), dt)
    for j, (s, n) in enumerate(bounds):
        out[:n, j] = M[s:s + n]
    return out.reshape(128, len(bounds) * W)


def _bsel():
    s = np.zeros((BL, BL * 128), np.float32)
    for b in range(BL):
        s[b, b * 128:(b + 1) * 128] = 1.0
    return s.astype(NPBF)


def _nu_mask(nu):
    D = np.zeros((NP_, BL, BL), np.float32)
    D[:, np.arange(BL), np.arange(BL)] = np.asarray(nu, np.float32)[:, None]
    return _fold128(D.reshape(NP_, B), NPBF)


def _pack_a(a, nC, L):
    """(BL, C, L) f32 -> (128, nC*BL*L) bf16, (j, b, l) free layout,
    zero-padded partial chunk."""
    C = a.shape[1]
    out = np.zeros((128, nC, BL, L), NPBF)
    for j in range(nC):
        kk = min(128, C - j * 128)
        out[:kk, j] = np.moveaxis(a[:, j * 128: j * 128 + kk], 0, 1)
    return out.reshape(128, nC * BL * L)


def _dup_outf(outfT, Lsubs):
    """(OF, L) out_f.T -> (128, 2*sum(2*ln)): each (o, sub) chunk
    duplicated side by side for batch-pair matmuls."""
    parts = []
    for o in range(2):
        chunk = outfT[o * 128:(o + 1) * 128]
        for (l0, l1) in Lsubs:
            c = chunk[:, l0:l1]
            parts.append(np.concatenate([c, c], axis=1))
    return np.ascontiguousarray(np.concatenate(parts, axis=1)).astype(NPBF)


def _prep_in_maps(inputs):
    f32 = np.float32

    def A(name):
        return np.asarray(inputs[name])

    x = A("x").reshape(B).astype(np.int64)
    hidden = A("hidden").astype(f32)          # (1, B, H)
    low = A("low_res").astype(f32).reshape(B, CL, LL)
    high = A("high_res").astype(f32).reshape(B, CH, LH)

    def outf(alpha, fc_w, fc_b, L):
        a = np.asarray(alpha, f32)
        o = np.asarray(fc_b, f32).reshape(L, OF)
        if np.any(a):
            o = o + (a.sum(0) @ np.asarray(fc_w, f32).T).reshape(L, OF)
        return np.ascontiguousarray(o.T)          # (OF, L) f32

    T = lambda name: np.asarray(inputs[name], f32).T

    def gru_w(w_name, b_name, bounds):
        wt = T(w_name)
        bias = A(b_name).astype(f32)[None]
        M = np.vstack([wt, bias])
        return _fold_blocks(M, bounds + [(M.shape[0] - 1, 1)], NPBF)

    rep = {
        "emb_m": _fold128(A("embedding").astype(f32), f32),
        "eye8": np.eye(BL, dtype=f32),
        "bsel": _bsel(),
        "g1ih_m": gru_w("gru1_w_ih", "gru1_b_ih", _chunks(EMB)),
        "g1hh_m": gru_w("gru1_w_hh", "gru1_b_hh", _chunks(H)),
        "g2ih_m": gru_w("gru2_w_ih", "gru2_b_ih", CTX_CH),
        "g2hh_m": gru_w("gru2_w_hh", "gru2_b_hh", _chunks(H)),
        "upT_low": _fold128(T("U_pred_low"), NPBF),
        "upT_high": _fold128(T("U_pred_high"), NPBF),
        "wT_low": np.concatenate([_fold128(T("U_a_low"), NPBF),
                                  _fold128(T("U_f_low"), NPBF)], axis=1),
        "wT_high": np.concatenate([_fold128(T("U_a_high"), NPBF),
                                   _fold128(T("U_f_high"), NPBF)], axis=1),
        "of_low": _dup_outf(outf(inputs["alpha_low"], inputs["fc_w_low"],
                                 inputs["fc_b_low"], LL), [(0, LL)]),
        "of_high": _dup_outf(outf(inputs["alpha_high"], inputs["fc_w_high"],
                                  inputs["fc_b_high"], LH), HSUB),
        "nu8_lo": _nu_mask(A("nu_low")),
        "nu8_hi": _nu_mask(A("nu_high")),
        "WsT_m": _fold128(T("W_s"), NPBF),
        "WcT_m": _fold_blocks(T("W_c"), CTX_CH, NPBF),
        "WoT_m": _fold128(T("W_o"), NPBF),
    }

    in_maps = []
    for c in range(NCORES):
        b0, b1 = c * BL, (c + 1) * BL
        sel = np.zeros((NCLS, BL), f32)
        sel[x[b0:b1], np.arange(BL)] = 1.0
        m = dict(rep)
        m["sel_m"] = _fold128(sel, f32)
        m["hT_m"] = _fold128(np.ascontiguousarray(hidden[0, b0:b1].T), NPBF)
        m["h_b"] = np.ascontiguousarray(hidden[0, b0:b1])
        m["A_low"] = _pack_a(low[b0:b1], 6, LL)
        m["A_high"] = _pack_a(high[b0:b1], 3, LH)
        in_maps.append(m)
    return in_maps


def _run(inputs, trace=False, trace_cores=None):
    if "nc" not in _STATE:
        _STATE["nc"] = _build_program()
    nc = _STATE["nc"]
    in_maps = _prep_in_maps(inputs)
    res = run_bass_kernel_spmd(nc, in_maps, list(range(NCORES)), trace=trace,
                               trace_cores=trace_cores)
    out = np.concatenate([res.results[c]["out"] for c in range(NCORES)], axis=0)
    new_h = np.concatenate([res.results[c]["new_h"] for c in range(NCORES)],
                           axis=0)[None]
    return (out.astype(np.float32), new_h.astype(np.float32)), res


def kernel(**inputs):
    (out, new_h), _ = _run(inputs)
    return out, new_h


# revision 8
# speedup vs baseline: 1.0829x; 1.0194x over previous
"""Trainium2 Bass kernel for a coverage-attention GRU decoder step.

Contract: kernel(**inputs) takes the full (unsharded) numpy inputs and
returns the full output, matching the reference:
    out (64, 144) f32, new_h (1, 64, 256) f32
Internally shards the batch (64) across 8 NeuronCores (8 per core),
replicating the (small) parameters.

Math note: the reference computes
    out_f = (alpha.sum(0) @ fc_w.T + fc_b).reshape(L, OF)
alpha_low/alpha_high are zero-filled by the problem spec, so the fc_w
term vanishes and out_f == fc_b.reshape(L, OF).  The host folds the
alpha-dependent term (exactly) only when alpha is nonzero; the device
kernel consumes out_f directly either way.

Layout notes (v2): all large operands are host-packed into SBUF-shaped
"mega" arrays — partition dim folded to 128 with zero padding, block
(j) major in the free dim — so each tensor is ONE contiguous DMA.
Activations are packed (j, b, l) so two batches are free-dim adjacent,
letting one N<=512 matmul compute a batch pair per weight load.
"""

import sys

try:
    import concourse  # noqa: F401
except ImportError:  # container stages the repo at /opt/trn_rl_repo
    sys.path.insert(0, "/opt/trn_rl_repo")

import numpy as np
import ml_dtypes

import concourse.bass as bass
import concourse.tile as tile
from concourse import bacc, mybir
from concourse.bass_utils import run_bass_kernel_spmd

F32 = mybir.dt.float32
BF16 = mybir.dt.bfloat16
NPBF = ml_dtypes.bfloat16

NCORES = 8
B = 64
BL = B // NCORES          # batches per core
EMB = 256
H = 256
NP_ = 512                 # attention projection dim
NCLS = 144
CL, CH = 684, 342
LL, LH = 256, 576
OF = 256
CTX = CL + CH

AX = mybir.AxisListType
AF = mybir.ActivationFunctionType
ALU = mybir.AluOpType


def _chunks(total, step=128):
    out = []
    s = 0
    while s < total:
        out.append((s, min(step, total - s)))
        s += step
    return out


C_LOW = _chunks(CL)       # 6 chunks (last 44 rows, zero-padded to 128)
C_HIGH = _chunks(CH)      # 3 chunks (last 86 rows)
# context rows: low chunks then high chunks (reference concatenates low,high)
CTX_CH = [(s, n) for (s, n) in C_LOW] + [(CL + s, n) for (s, n) in C_HIGH]
# high L split: pairs of these fit one PSUM bank (N = 2*ln <= 512)
HSUB = [(0, 256), (256, 512), (512, 576)]

_STATE = {}


# --------------------------------------------------------------------------
# device program
# --------------------------------------------------------------------------

def _build_program():
    nc = bacc.Bacc("TRN2", target_bir_lowering=False, debug=False,
                   enable_asserts=False, num_devices=NCORES)

    def din(name, shape, dt=F32):
        return nc.dram_tensor(name, shape, dt, kind="ExternalInput").ap()

    d = {}
    # per-core data (host-packed)
    d["sel_m"] = din("sel_m", [128, 2 * BL])             # one-hot, folded
    d["hT_m"] = din("hT_m", [128, 2 * BL], BF16)         # hidden.T folded
    d["h_b"] = din("h_b", [BL, H])
    d["A_low"] = din("A_low", [128, 6 * BL * LL], BF16)  # (j,b,l) packed
    d["A_high"] = din("A_high", [128, 3 * BL * LH], BF16)
    # replicated params (host-packed)
    d["emb_m"] = din("emb_m", [128, 2 * EMB])            # embedding folded f32
    d["eye8"] = din("eye8", [BL, BL])
    d["bsel"] = din("bsel", [BL, BL * 128], BF16)        # row-broadcast sel
    d["g1ih_m"] = din("g1ih_m", [128, 3 * 3 * H], BF16)  # [w.T;b] folded
    d["g1hh_m"] = din("g1hh_m", [128, 3 * 3 * H], BF16)
    d["g2ih_m"] = din("g2ih_m", [128, 10 * 3 * H], BF16)  # ctx-chunk blocks
    d["g2hh_m"] = din("g2hh_m", [128, 3 * 3 * H], BF16)
    d["upT_low"] = din("upT_low", [128, 2 * NP_], BF16)
    d["upT_high"] = din("upT_high", [128, 2 * NP_], BF16)
    d["wT_low"] = din("wT_low", [128, 8 * NP_], BF16)    # [U_a.T | U_f.T]
    d["wT_high"] = din("wT_high", [128, 5 * NP_], BF16)
    d["of_low"] = din("of_low", [128, 2 * 2 * LL], BF16)   # out_f.T dup'd
    d["of_high"] = din("of_high", [128, 2 * 2 * LH], BF16)
    d["nu8_lo"] = din("nu8_lo", [128, 4 * B], BF16)      # nu-masked, folded
    d["nu8_hi"] = din("nu8_hi", [128, 4 * B], BF16)
    d["WsT_m"] = din("WsT_m", [128, 2 * EMB], BF16)
    d["WcT_m"] = din("WcT_m", [128, 9 * EMB], BF16)
    d["WoT_m"] = din("WoT_m", [128, 2 * NCLS], BF16)

    out_d = nc.dram_tensor("out", [BL, NCLS], F32, kind="ExternalOutput").ap()
    newh_d = nc.dram_tensor("new_h", [BL, H], F32, kind="ExternalOutput").ap()

    from contextlib import ExitStack
    with tile.TileContext(nc) as tc, ExitStack() as ctx:
        P = ctx.enter_context(tc.tile_pool(name="persist", bufs=1))
        ST = ctx.enter_context(tc.tile_pool(name="stage", bufs=4))
        TH = ctx.enter_context(tc.tile_pool(name="tanhp", bufs=16))
        PSB = ctx.enter_context(tc.tile_pool(name="psbig", bufs=5, space="PSUM"))
        PSS = ctx.enter_context(tc.tile_pool(name="pssm", bufs=3, space="PSUM"))

        def ptile(shape, dt, tag):
            return P.tile(shape, dt, tag=tag, name=tag)

        def load(name, shape, dt):
            t = ptile(shape, dt, name)
            nc.sync.dma_start(t[:], d[name][:])
            return t

        # ------------- persistent loads (small first; A last, chunked) -----
        sel_m = load("sel_m", [128, 2 * BL], F32)
        emb_m = load("emb_m", [128, 2 * EMB], F32)
        hT_m = load("hT_m", [128, 2 * BL], BF16)
        h_b = load("h_b", [BL, H], F32)
        eye8 = load("eye8", [BL, BL], F32)
        bsel = load("bsel", [BL, BL * 128], BF16)
        g1ih_m = load("g1ih_m", [128, 9 * H], BF16)
        g1hh_m = load("g1hh_m", [128, 9 * H], BF16)
        upT = {"lo": load("upT_low", [128, 2 * NP_], BF16),
               "hi": load("upT_high", [128, 2 * NP_], BF16)}
        nu8 = {"lo": load("nu8_lo", [128, 4 * B], BF16),
               "hi": load("nu8_hi", [128, 4 * B], BF16)}
        wT = {"lo": load("wT_low", [128, 8 * NP_], BF16),
              "hi": load("wT_high", [128, 5 * NP_], BF16)}
        of = {"lo": load("of_low", [128, 4 * LL], BF16)}
        of["hi"] = ptile([128, 4 * LH], BF16, "of_high")
        nc.scalar.dma_start(of["hi"][:], d["of_high"][:])
        # A tiles: one DMA per k-chunk so matmuls start on chunk arrival;
        # A_high + its weights ride the second HWDGE ring (scalar).
        A_low = ptile([128, 6 * BL * LL], BF16, "A_low")
        for j in range(6):
            nc.sync.dma_start(A_low[:, j * BL * LL:(j + 1) * BL * LL],
                              d["A_low"][:, j * BL * LL:(j + 1) * BL * LL])
        A_high = ptile([128, 3 * BL * LH], BF16, "A_high")
        for j in range(3):
            nc.scalar.dma_start(A_high[:, j * BL * LH:(j + 1) * BL * LH],
                                d["A_high"][:, j * BL * LH:(j + 1) * BL * LH])
        # needed only in phase C — queue behind the A tiles
        g2ih_m = load("g2ih_m", [128, 30 * H], BF16)
        g2hh_m = load("g2hh_m", [128, 9 * H], BF16)
        WsT_m = load("WsT_m", [128, 2 * EMB], BF16)
        WcT_m = load("WcT_m", [128, 9 * EMB], BF16)
        WoT_m = load("WoT_m", [128, 2 * NCLS], BF16)

        ones8 = ptile([1, BL], BF16, "ones8")
        nc.vector.memset(ones8[:], 1.0)

        # ---------------- phase A: gather + gru1 ----------------
        # embedded.T = embedding.T @ onehot  (f32, exact)
        embT_f32, embT_bf = [], []
        for mi in range(2):
            ps = PSS.tile([128, BL], F32, tag="sm", name="sm")
            for j in range(2):
                nc.tensor.matmul(ps[:], emb_m[:, j * EMB + mi * 128:
                                              j * EMB + mi * 128 + 128],
                                 sel_m[:, j * BL:(j + 1) * BL],
                                 start=(j == 0), stop=(j == 1))
            ef = ptile([128, BL], F32, f"embTf{mi}")
            nc.vector.tensor_copy(ef[:], ps[:])
            eb = ptile([128, BL], BF16, f"embTb{mi}")
            nc.vector.tensor_copy(eb[:], ps[:])
            embT_f32.append(ef)
            embT_bf.append(eb)

        def gru(lh_ih, lh_hh, wih_m, whh_m, h_tile, tag):
            """lh_*: stationary (tile-ap, K) lists; last entry is the bias
            (ones8, 1).  wih_m/whh_m: folded moving weights whose j-th
            128-row block pairs with lh[j]; the bias row is the last
            block's row 0.  Returns (BL, H) f32 new hidden."""
            ps_r = PSS.tile([BL, H], F32, tag="sm", name="sm")
            ps_z = PSS.tile([BL, H], F32, tag="sm", name="sm")
            ps_in = PSS.tile([BL, H], F32, tag="sm", name="sm")
            ps_hn = PSS.tile([BL, H], F32, tag="sm", name="sm")

            def acc(ps, col0, sides):
                steps = []
                for lhs, mov in sides:
                    for j, (lt, lk) in enumerate(lhs):
                        steps.append((lt, lk, mov, j))
                for i, (lt, lk, mov, j) in enumerate(steps):
                    nc.tensor.matmul(
                        ps[:], lt[:lk, :],
                        mov[:lk, j * 3 * H + col0: j * 3 * H + col0 + H],
                        start=(i == 0), stop=(i == len(steps) - 1))

            acc(ps_r, 0, [(lh_ih, wih_m), (lh_hh, whh_m)])
            acc(ps_z, H, [(lh_ih, wih_m), (lh_hh, whh_m)])
            acc(ps_in, 2 * H, [(lh_ih, wih_m)])
            acc(ps_hn, 2 * H, [(lh_hh, whh_m)])

            r = ST.tile([BL, H], F32, tag=f"{tag}r", name=f"{tag}r", bufs=1)
            nc.scalar.activation(r[:], ps_r[:], AF.Sigmoid)
            z = ST.tile([BL, H], F32, tag=f"{tag}z", name=f"{tag}z", bufs=1)
            nc.scalar.activation(z[:], ps_z[:], AF.Sigmoid)
            rh = ST.tile([BL, H], F32, tag=f"{tag}rh", name=f"{tag}rh", bufs=1)
            nc.vector.tensor_tensor(rh[:], r[:], ps_hn[:], ALU.mult)
            npre = ST.tile([BL, H], F32, tag=f"{tag}np", name=f"{tag}np",
                           bufs=1)
            nc.vector.tensor_tensor(npre[:], rh[:], ps_in[:], ALU.add)
            n = ST.tile([BL, H], F32, tag=f"{tag}n", name=f"{tag}n", bufs=1)
            nc.scalar.activation(n[:], npre[:], AF.Tanh)
            dif = ST.tile([BL, H], F32, tag=f"{tag}d", name=f"{tag}d", bufs=1)
            nc.vector.tensor_tensor(dif[:], h_tile[:], n[:], ALU.subtract)
            zd = ST.tile([BL, H], F32, tag=f"{tag}zd", name=f"{tag}zd", bufs=1)
            nc.vector.tensor_tensor(zd[:], z[:], dif[:], ALU.mult)
            nh = ptile([BL, H], F32, f"{tag}_out")
            nc.vector.tensor_tensor(nh[:], n[:], zd[:], ALU.add)
            return nh

        eT = [(embT_bf[0], 128), (embT_bf[1], 128), (ones8, 1)]
        hT = [(hT_m[:, 0:BL], 128), (hT_m[:, BL:2 * BL], 128), (ones8, 1)]
        pred = gru(eT, hT, g1ih_m, g1hh_m, h_b, "g1")

        # pred.T in bf16 (2 chunks of (128, BL))
        predT = []
        for ki in range(2):
            ps = PSS.tile([128, BL], F32, tag="sm", name="sm")
            nc.tensor.transpose(ps[:], pred[:, ki * 128:(ki + 1) * 128],
                                eye8[:])
            t = ptile([128, BL], BF16, f"predT{ki}")
            nc.vector.tensor_copy(t[:], ps[:])
            predT.append(t)

        # u_pred.T per block: 4 chunks of (128, BL) f32 (ACT bias columns)
        up_sb = {}
        for blk in ("lo", "hi"):
            res = []
            for mi in range(4):
                ps = PSS.tile([128, BL], F32, tag="sm", name="sm")
                for j in range(2):
                    nc.tensor.matmul(
                        ps[:], upT[blk][:, j * NP_ + mi * 128:
                                        j * NP_ + mi * 128 + 128],
                        predT[j][:], start=(j == 0), stop=(j == 1))
                t = ptile([128, BL], F32, f"up{blk}{mi}")
                nc.vector.tensor_copy(t[:], ps[:])
                res.append(t)
            up_sb[blk] = res

        # ---------------- phase B: coverage attention ----------------
        def block(blk, nC, L, Lsubs, A_t, nK):
            """nK = nC a-blocks + 2 out_f blocks.  Returns ctxT_bf chunks."""
            Av = A_t.rearrange("p (x l) -> p x l", l=L)   # (128, nC*BL, L)
            pe = [PSS.tile([BL, l1 - l0], F32, tag="sm", name="sm")
                  for (l0, l1) in Lsubs]
            for mi in range(4):
                ths = [TH.tile([128, L], BF16, tag="tanh", name="tanh")
                       for _ in range(BL)]
                for si, (l0, l1) in enumerate(Lsubs):
                    ln = l1 - l0
                    pts = [PSB.tile([128, 512], F32, tag="pt", name="pt")
                           for _ in range(4)]
                    for kc in range(nK):
                        w = wT[blk][:, kc * NP_ + mi * 128:
                                    kc * NP_ + mi * 128 + 128]
                        for q in range(4):
                            if kc < nC:
                                rhs = Av[:, kc * BL + 2 * q:
                                         kc * BL + 2 * q + 2, l0:l1]
                            else:
                                base = (kc - nC) * 2 * L + 2 * l0
                                rhs = of[blk][:, base: base + 2 * ln]
                            nc.tensor.matmul(pts[q][:, 0:2 * ln], w, rhs,
                                             start=(kc == 0),
                                             stop=(kc == nK - 1))
                    for b in range(BL):
                        nc.scalar.activation(
                            ths[b][:, l0:l1],
                            pts[b // 2][:, (b % 2) * ln:(b % 2 + 1) * ln],
                            AF.Tanh, bias=up_sb[blk][mi][:, b:b + 1])
                for b in range(BL):
                    for si, (l0, l1) in enumerate(Lsubs):
                        nc.tensor.matmul(
                            pe[si][:, :],
                            nu8[blk][:, mi * B + b * BL: mi * B + (b + 1) * BL],
                            ths[b][:, l0:l1],
                            start=(mi == 0 and b == 0),
                            stop=(mi == 3 and b == BL - 1))

            # softmax over L (rows = batches)
            et = ST.tile([BL, L], F32, tag="et", name="et", bufs=2)
            for si, (l0, l1) in enumerate(Lsubs):
                nc.vector.tensor_copy(et[:, l0:l1], pe[si][:, :])
            nmax = ST.tile([BL, 1], F32, tag="nmax", name="nmax")
            nc.vector.tensor_reduce(nmax[:], et[:], axis=AX.X, op=ALU.max,
                                    negate=True)
            expt = ST.tile([BL, L], BF16, tag="expt", name="expt", bufs=2)
            sums = ST.tile([BL, 1], F32, tag="sums", name="sums")
            nc.scalar.activation(expt[:], et[:], AF.Exp, bias=nmax[:],
                                 accum_out=sums[:])
            recip = ST.tile([BL, 1], F32, tag="recip", name="recip")
            nc.vector.reciprocal(recip[:], sums[:])
            alph = ST.tile([BL, L], BF16, tag="alph", name="alph", bufs=2)
            nc.vector.tensor_scalar_mul(alph[:], expt[:], recip[:])

            # ctx.T chunks (c on partitions, batch on free); padded rows
            # of A are zero so they accumulate exact zeros.
            abs_ = []
            for b in range(BL):
                ab = ST.tile([128, L], BF16, tag="ab", name="ab", bufs=8)
                for si, (l0, l1) in enumerate(Lsubs):
                    ln = l1 - l0
                    abp = PSS.tile([128, 512], F32, tag="sm", name="sm")
                    nc.tensor.matmul(abp[:, :ln],
                                     bsel[:, b * 128:(b + 1) * 128],
                                     alph[:, l0:l1], start=True, stop=True)
                    nc.vector.tensor_copy(ab[:, l0:l1], abp[:, :ln])
                abs_.append(ab)
            res = []
            for ci in range(nC):
                ctx_f = ptile([128, BL], F32, f"{blk}ctx{ci}")
                for b in range(BL):
                    junk = ST.tile([128, L], BF16, tag="junk", name="junk",
                                   bufs=4)
                    nc.vector.scalar_tensor_tensor(
                        junk[:], Av[:, ci * BL + b, :], 1.0, abs_[b][:],
                        ALU.mult, ALU.mult,
                        accum_out=ctx_f[:, b:b + 1])
                t = ptile([128, BL], BF16, f"{blk}ctxb{ci}")
                nc.vector.tensor_copy(t[:], ctx_f[:])
                res.append(t)
            return res

        ctx_low = block("lo", 6, LL, [(0, LL)], A_low, 8)
        ctx_high = block("hi", 3, LH, HSUB, A_high, 5)
        ctxT = ctx_low + ctx_high            # 9 chunks, zero-padded rows

        # ---------------- phase C: gru2 + output ----------------
        lh_ih2 = [(t, 128) for t in ctxT] + [(ones8, 1)]
        lh_hh2 = [(predT[0], 128), (predT[1], 128), (ones8, 1)]
        new_h = gru(lh_ih2, lh_hh2, g2ih_m, g2hh_m, pred, "g2")
        nc.sync.dma_start(newh_d[:], new_h[:])

        # new_h.T bf16
        nhT = []
        for ki in range(2):
            ps = PSS.tile([128, BL], F32, tag="sm", name="sm")
            nc.tensor.transpose(ps[:], new_h[:, ki * 128:(ki + 1) * 128],
                                eye8[:])
            t = ptile([128, BL], BF16, f"nhT{ki}")
            nc.vector.tensor_copy(t[:], ps[:])
            nhT.append(t)

        # S.T = embedded.T + W_s @ new_h.T + W_c @ ctx.T   (per EMB chunk)
        sT = []
        for mi in range(2):
            ps = PSS.tile([128, BL], F32, tag="sm", name="sm")
            steps = [(WsT_m[:, j * EMB + mi * 128: j * EMB + mi * 128 + 128],
                      nhT[j]) for j in range(2)]
            steps += [(WcT_m[:, j * EMB + mi * 128: j * EMB + mi * 128 + 128],
                       ctxT[j]) for j in range(9)]
            for i, (w, r) in enumerate(steps):
                nc.tensor.matmul(ps[:], w, r[:], start=(i == 0),
                                 stop=(i == len(steps) - 1))
            t = ptile([128, BL], BF16, f"sT{mi}")
            nc.vector.tensor_tensor(t[:], ps[:], embT_f32[mi][:], ALU.add)
            sT.append(t)

        # out = S @ W_o.T : (BL, NCLS)
        ps = PSS.tile([BL, NCLS], F32, tag="sm", name="sm")
        for ki in range(2):
            nc.tensor.matmul(ps[:], sT[ki][:],
                             WoT_m[:, ki * NCLS:(ki + 1) * NCLS],
                             start=(ki == 0), stop=(ki == 1))
        out_sb = ST.tile([BL, NCLS], F32, tag="outsb", name="outsb")
        nc.vector.tensor_copy(out_sb[:], ps[:])
        nc.sync.dma_start(out_d[:], out_sb[:])

    nc.compile()
    return nc


# --------------------------------------------------------------------------
# host side
# --------------------------------------------------------------------------

def _fold128(M, dt):
    """(R, W) -> (128, ceil(R/128)*W): 128-row blocks along the free dim,
    zero padded."""
    R, W = M.shape
    nj = (R + 127) // 128
    out = np.zeros((128, nj, W), dt)
    for j in range(nj):
        kk = min(128, R - j * 128)
        out[:kk, j] = M[j * 128: j * 128 + kk]
    return out.reshape(128, nj * W)


def _fold_blocks(M, bounds, dt):
    """rows of M grouped by (start, size) bounds, each padded to 128."""
    W = M.shape[1]
    out = np.zeros((128, len(bounds), W), dt)
    for j, (s, n) in enumerate(bounds):
        out[:n, j] = M[s:s + n]
    return out.reshape(128, len(bounds) * W)


def _bsel():
    s = np.zeros((BL, BL * 128), np.float32)
    for b in range(BL):
        s[b, b * 128:(b + 1) * 128] = 1.0
    return s.astype(NPBF)


def _nu_mask(nu):
    D = np.zeros((NP_, BL, BL), np.float32)
    D[:, np.arange(BL), np.arange(BL)] = np.asarray(nu, np.float32)[:, None]
    return _fold128(D.reshape(NP_, B), NPBF)


def _pack_a(a, nC, L):
    """(BL, C, L) f32 -> (128, nC*BL*L) bf16, (j, b, l) free layout,
    zero-padded partial chunk."""
    C = a.shape[1]
    out = np.zeros((128, nC, BL, L), NPBF)
    for j in range(nC):
        kk = min(128, C - j * 128)
        out[:kk, j] = np.moveaxis(a[:, j * 128: j * 128 + kk], 0, 1)
    return out.reshape(128, nC * BL * L)


def _dup_outf(outfT, Lsubs):
    """(OF, L) out_f.T -> (128, 2*sum(2*ln)): each (o, sub) chunk
    duplicated side by side for batch-pair matmuls."""
    parts = []
    for o in range(2):
        chunk = outfT[o * 128:(o + 1) * 128]
        for (l0, l1) in Lsubs:
            c = chunk[:, l0:l1]
            parts.append(np.concatenate([c, c], axis=1))
    return np.ascontiguousarray(np.concatenate(parts, axis=1)).astype(NPBF)


def _prep_in_maps(inputs):
    f32 = np.float32

    def A(name):
        return np.asarray(inputs[name])

    x = A("x").reshape(B).astype(np.int64)
    hidden = A("hidden").astype(f32)          # (1, B, H)
    low = A("low_res").astype(f32).reshape(B, CL, LL)
    high = A("high_res").astype(f32).reshape(B, CH, LH)

    def outf(alpha, fc_w, fc_b, L):
        a = np.asarray(alpha, f32)
        o = np.asarray(fc_b, f32).reshape(L, OF)
        if np.any(a):
            o = o + (a.sum(0) @ np.asarray(fc_w, f32).T).reshape(L, OF)
        return np.ascontiguousarray(o.T)          # (OF, L) f32

    T = lambda name: np.asarray(inputs[name], f32).T

    def gru_w(w_name, b_name, bounds):
        wt = T(w_name)
        bias = A(b_name).astype(f32)[None]
        M = np.vstack([wt, bias])
        return _fold_blocks(M, bounds + [(M.shape[0] - 1, 1)], NPBF)

    rep = {
        "emb_m": _fold128(A("embedding").astype(f32), f32),
        "eye8": np.eye(BL, dtype=f32),
        "bsel": _bsel(),
        "g1ih_m": gru_w("gru1_w_ih", "gru1_b_ih", _chunks(EMB)),
        "g1hh_m": gru_w("gru1_w_hh", "gru1_b_hh", _chunks(H)),
        "g2ih_m": gru_w("gru2_w_ih", "gru2_b_ih", CTX_CH),
        "g2hh_m": gru_w("gru2_w_hh", "gru2_b_hh", _chunks(H)),
        "upT_low": _fold128(T("U_pred_low"), NPBF),
        "upT_high": _fold128(T("U_pred_high"), NPBF),
        "wT_low": np.concatenate([_fold128(T("U_a_low"), NPBF),
                                  _fold128(T("U_f_low"), NPBF)], axis=1),
        "wT_high": np.concatenate([_fold128(T("U_a_high"), NPBF),
                                   _fold128(T("U_f_high"), NPBF)], axis=1),
        "of_low": _dup_outf(outf(inputs["alpha_low"], inputs["fc_w_low"],
                                 inputs["fc_b_low"], LL), [(0, LL)]),
        "of_high": _dup_outf(outf(inputs["alpha_high"], inputs["fc_w_high"],
                                  inputs["fc_b_high"], LH), HSUB),
        "nu8_lo": _nu_mask(A("nu_low")),
        "nu8_hi": _nu_mask(A("nu_high")),
        "WsT_m": _fold128(T("W_s"), NPBF),
        "WcT_m": _fold_blocks(T("W_c"), CTX_CH, NPBF),
        "WoT_m": _fold128(T("W_o"), NPBF),
    }

    in_maps = []
    for c in range(NCORES):
        b0, b1 = c * BL, (c + 1) * BL
        sel = np.zeros((NCLS, BL), f32)
        sel[x[b0:b1], np.arange(BL)] = 1.0
        m = dict(rep)
        m["sel_m"] = _fold128(sel, f32)
        m["hT_m"] = _fold128(np.ascontiguousarray(hidden[0, b0:b1].T), NPBF)
        m["h_b"] = np.ascontiguousarray(hidden[0, b0:b1])
        m["A_low"] = _pack_a(low[b0:b1], 6, LL)
        m["A_high"] = _pack_a(high[b0:b1], 3, LH)
        in_maps.append(m)
    return in_maps


def _run(inputs, trace=False, trace_cores=None):
    if "nc" not in _STATE:
        _STATE["nc"] = _build_program()
    nc = _STATE["nc"]
    in_maps = _prep_in_maps(inputs)
    res = run_bass_kernel_spmd(nc, in_maps, list(range(NCORES)), trace=trace,
                               trace_cores=trace_cores)
    out = np.concatenate([res.results[c]["out"] for c in range(NCORES)], axis=0)
    new_h = np.concatenate([res.results[c]["new_h"] for c in range(NCORES)],
                           axis=0)[None]
    return (out.astype(np.float32), new_h.astype(np.float32)), res


def kernel(**inputs):
    (out, new_h), _ = _run(inputs)
    return out, new_h


# revision 10
# speedup vs baseline: 1.1181x; 1.0325x over previous
"""Trainium2 Bass kernel for a coverage-attention GRU decoder step.

Contract: kernel(**inputs) takes the full (unsharded) numpy inputs and
returns the full output, matching the reference:
    out (64, 144) f32, new_h (1, 64, 256) f32
Internally shards the batch (64) across 8 NeuronCores (8 per core),
replicating the (small) parameters.

Math note: the reference computes
    out_f = (alpha.sum(0) @ fc_w.T + fc_b).reshape(L, OF)
alpha_low/alpha_high are zero-filled by the problem spec, so the fc_w
term vanishes and out_f == fc_b.reshape(L, OF).  The host folds the
alpha-dependent term (exactly) only when alpha is nonzero; the device
kernel consumes out_f directly either way.

Layout notes (v2): all large operands are host-packed into SBUF-shaped
"mega" arrays — partition dim folded to 128 with zero padding, block
(j) major in the free dim — so each tensor is ONE contiguous DMA.
Activations are packed (j, b, l) so two batches are free-dim adjacent,
letting one N<=512 matmul compute a batch pair per weight load.
"""

import sys

try:
    import concourse  # noqa: F401
except ImportError:  # container stages the repo at /opt/trn_rl_repo
    sys.path.insert(0, "/opt/trn_rl_repo")

import numpy as np
import ml_dtypes

import concourse.bass as bass
import concourse.tile as tile
from concourse import bacc, mybir
from concourse.bass_utils import run_bass_kernel_spmd

F32 = mybir.dt.float32
BF16 = mybir.dt.bfloat16
NPBF = ml_dtypes.bfloat16

NCORES = 8
B = 64
BL = B // NCORES          # batches per core
EMB = 256
H = 256
NP_ = 512                 # attention projection dim
NCLS = 144
CL, CH = 684, 342
LL, LH = 256, 576
OF = 256
CTX = CL + CH

AX = mybir.AxisListType
AF = mybir.ActivationFunctionType
ALU = mybir.AluOpType


def _chunks(total, step=128):
    out = []
    s = 0
    while s < total:
        out.append((s, min(step, total - s)))
        s += step
    return out


C_LOW = _chunks(CL)       # 6 chunks (last 44 rows, zero-padded to 128)
C_HIGH = _chunks(CH)      # 3 chunks (last 86 rows)
# context rows: low chunks then high chunks (reference concatenates low,high)
CTX_CH = [(s, n) for (s, n) in C_LOW] + [(CL + s, n) for (s, n) in C_HIGH]
# high L split: pairs of these fit one PSUM bank (N = 2*ln <= 512)
HSUB = [(0, 256), (256, 512), (512, 576)]

_STATE = {}


# --------------------------------------------------------------------------
# device program
# --------------------------------------------------------------------------

def _build_program():
    nc = bacc.Bacc("TRN2", target_bir_lowering=False, debug=False,
                   enable_asserts=False, num_devices=NCORES)

    def din(name, shape, dt=F32):
        return nc.dram_tensor(name, shape, dt, kind="ExternalInput").ap()

    d = {}
    # per-core data (host-packed)
    d["sel_m"] = din("sel_m", [128, 2 * BL])             # one-hot, folded
    d["hT_m"] = din("hT_m", [128, 2 * BL], BF16)         # hidden.T folded
    d["h_b"] = din("h_b", [BL, H])
    d["A_low"] = din("A_low", [128, 6 * BL * LL], BF16)  # (j,b,l) packed
    d["A_high"] = din("A_high", [128, 3 * BL * LH], BF16)
    # replicated params (host-packed)
    d["emb_m"] = din("emb_m", [128, 2 * EMB])            # embedding folded f32
    d["eye8"] = din("eye8", [BL, BL])
    d["bsel"] = din("bsel", [BL, BL * 128], BF16)        # row-broadcast sel
    d["g1ih_m"] = din("g1ih_m", [128, 3 * 3 * H], BF16)  # [w.T;b] folded
    d["g1hh_m"] = din("g1hh_m", [128, 3 * 3 * H], BF16)
    d["g2ih_m"] = din("g2ih_m", [128, 10 * 3 * H], BF16)  # ctx-chunk blocks
    d["g2hh_m"] = din("g2hh_m", [128, 3 * 3 * H], BF16)
    d["upT_low"] = din("upT_low", [128, 2 * NP_], BF16)
    d["upT_high"] = din("upT_high", [128, 2 * NP_], BF16)
    d["wT_low"] = din("wT_low", [128, 8 * NP_], BF16)    # [U_a.T | U_f.T]
    d["wT_high"] = din("wT_high", [128, 5 * NP_], BF16)
    d["of_low"] = din("of_low", [128, 2 * 2 * LL], BF16)   # out_f.T dup'd
    d["of_high"] = din("of_high", [128, 2 * 2 * LH], BF16)
    d["nu8_lo"] = din("nu8_lo", [128, 4 * B], BF16)      # nu-masked, folded
    d["nu8_hi"] = din("nu8_hi", [128, 4 * B], BF16)
    d["WsT_m"] = din("WsT_m", [128, 2 * EMB], BF16)
    d["WcT_m"] = din("WcT_m", [128, 9 * EMB], BF16)
    d["WoT_m"] = din("WoT_m", [128, 2 * NCLS], BF16)

    out_d = nc.dram_tensor("out", [BL, NCLS], F32, kind="ExternalOutput").ap()
    newh_d = nc.dram_tensor("new_h", [BL, H], F32, kind="ExternalOutput").ap()

    from contextlib import ExitStack
    with tile.TileContext(nc) as tc, ExitStack() as ctx:
        P = ctx.enter_context(tc.tile_pool(name="persist", bufs=1))
        ST = ctx.enter_context(tc.tile_pool(name="stage", bufs=4))
        TH = ctx.enter_context(tc.tile_pool(name="tanhp", bufs=16))
        PSB = ctx.enter_context(tc.tile_pool(name="psbig", bufs=5, space="PSUM"))
        PSS = ctx.enter_context(tc.tile_pool(name="pssm", bufs=3, space="PSUM"))

        def ptile(shape, dt, tag):
            return P.tile(shape, dt, tag=tag, name=tag)

        def load(name, shape, dt):
            t = ptile(shape, dt, name)
            nc.sync.dma_start(t[:], d[name][:])
            return t

        # ------------- persistent loads --------------------------------
        # sync ring: gather + low-block feed, then phase-C weights.
        # scalar ring: phase-A smalls + high-block feed.
        def load2(name, shape, dt):
            t = ptile(shape, dt, name)
            nc.scalar.dma_start(t[:], d[name][:])
            return t

        sel_m = load("sel_m", [128, 2 * BL], F32)
        emb_m = load("emb_m", [128, 2 * EMB], F32)
        wT = {"lo": load("wT_low", [128, 8 * NP_], BF16)}
        of = {"lo": load("of_low", [128, 4 * LL], BF16)}
        A_low = ptile([128, 6 * BL * LL], BF16, "A_low")
        for j in range(6):
            nc.sync.dma_start(A_low[:, j * BL * LL:(j + 1) * BL * LL],
                              d["A_low"][:, j * BL * LL:(j + 1) * BL * LL])
        bsel = load("bsel", [BL, BL * 128], BF16)
        g2ih_m = load("g2ih_m", [128, 30 * H], BF16)
        g2hh_m = load("g2hh_m", [128, 9 * H], BF16)
        WsT_m = load("WsT_m", [128, 2 * EMB], BF16)
        WcT_m = load("WcT_m", [128, 9 * EMB], BF16)
        WoT_m = load("WoT_m", [128, 2 * NCLS], BF16)

        nu8 = {"lo": load2("nu8_lo", [128, 4 * B], BF16),
               "hi": load2("nu8_hi", [128, 4 * B], BF16)}
        hT_m = load2("hT_m", [128, 2 * BL], BF16)
        h_b = load2("h_b", [BL, H], F32)
        eye8 = load2("eye8", [BL, BL], F32)
        g1ih_m = load2("g1ih_m", [128, 9 * H], BF16)
        g1hh_m = load2("g1hh_m", [128, 9 * H], BF16)
        upT = {"lo": load2("upT_low", [128, 2 * NP_], BF16),
               "hi": load2("upT_high", [128, 2 * NP_], BF16)}
        wT["hi"] = load2("wT_high", [128, 5 * NP_], BF16)
        of["hi"] = load2("of_high", [128, 4 * LH], BF16)
        A_high = ptile([128, 3 * BL * LH], BF16, "A_high")
        for j in range(3):
            nc.scalar.dma_start(A_high[:, j * BL * LH:(j + 1) * BL * LH],
                                d["A_high"][:, j * BL * LH:(j + 1) * BL * LH])

        ones8 = ptile([1, BL], BF16, "ones8")
        nc.vector.memset(ones8[:], 1.0)

        # ---------------- phase A: gather + gru1 ----------------
        # embedded.T = embedding.T @ onehot  (f32, exact)
        embT_f32, embT_bf = [], []
        for mi in range(2):
            ps = PSS.tile([128, BL], F32, tag="sm", name="sm")
            for j in range(2):
                nc.tensor.matmul(ps[:], emb_m[:, j * EMB + mi * 128:
                                              j * EMB + mi * 128 + 128],
                                 sel_m[:, j * BL:(j + 1) * BL],
                                 start=(j == 0), stop=(j == 1))
            ef = ptile([128, BL], F32, f"embTf{mi}")
            nc.vector.tensor_copy(ef[:], ps[:])
            eb = ptile([128, BL], BF16, f"embTb{mi}")
            nc.vector.tensor_copy(eb[:], ps[:])
            embT_f32.append(ef)
            embT_bf.append(eb)

        def gru(lh_ih, lh_hh, wih_m, whh_m, h_tile, tag):
            """lh_*: stationary (tile-ap, K) lists; last entry is the bias
            (ones8, 1).  wih_m/whh_m: folded moving weights whose j-th
            128-row block pairs with lh[j]; the bias row is the last
            block's row 0.  Returns (BL, H) f32 new hidden."""
            ps_r = PSS.tile([BL, H], F32, tag="sm", name="sm")
            ps_z = PSS.tile([BL, H], F32, tag="sm", name="sm")
            ps_in = PSS.tile([BL, H], F32, tag="sm", name="sm")
            ps_hn = PSS.tile([BL, H], F32, tag="sm", name="sm")

            def acc(ps, col0, sides):
                steps = []
                for lhs, mov in sides:
                    for j, (lt, lk) in enumerate(lhs):
                        steps.append((lt, lk, mov, j))
                for i, (lt, lk, mov, j) in enumerate(steps):
                    nc.tensor.matmul(
                        ps[:], lt[:lk, :],
                        mov[:lk, j * 3 * H + col0: j * 3 * H + col0 + H],
                        start=(i == 0), stop=(i == len(steps) - 1))

            acc(ps_r, 0, [(lh_ih, wih_m), (lh_hh, whh_m)])
            acc(ps_z, H, [(lh_ih, wih_m), (lh_hh, whh_m)])
            acc(ps_in, 2 * H, [(lh_ih, wih_m)])
            acc(ps_hn, 2 * H, [(lh_hh, whh_m)])

            r = ST.tile([BL, H], F32, tag=f"{tag}r", name=f"{tag}r", bufs=1)
            nc.scalar.activation(r[:], ps_r[:], AF.Sigmoid)
            z = ST.tile([BL, H], F32, tag=f"{tag}z", name=f"{tag}z", bufs=1)
            nc.scalar.activation(z[:], ps_z[:], AF.Sigmoid)
            rh = ST.tile([BL, H], F32, tag=f"{tag}rh", name=f"{tag}rh", bufs=1)
            nc.vector.tensor_tensor(rh[:], r[:], ps_hn[:], ALU.mult)
            npre = ST.tile([BL, H], F32, tag=f"{tag}np", name=f"{tag}np",
                           bufs=1)
            nc.vector.tensor_tensor(npre[:], rh[:], ps_in[:], ALU.add)
            n = ST.tile([BL, H], F32, tag=f"{tag}n", name=f"{tag}n", bufs=1)
            nc.scalar.activation(n[:], npre[:], AF.Tanh)
            dif = ST.tile([BL, H], F32, tag=f"{tag}d", name=f"{tag}d", bufs=1)
            nc.vector.tensor_tensor(dif[:], h_tile[:], n[:], ALU.subtract)
            zd = ST.tile([BL, H], F32, tag=f"{tag}zd", name=f"{tag}zd", bufs=1)
            nc.vector.tensor_tensor(zd[:], z[:], dif[:], ALU.mult)
            nh = ptile([BL, H], F32, f"{tag}_out")
            nc.vector.tensor_tensor(nh[:], n[:], zd[:], ALU.add)
            return nh

        eT = [(embT_bf[0], 128), (embT_bf[1], 128), (ones8, 1)]
        hT = [(hT_m[:, 0:BL], 128), (hT_m[:, BL:2 * BL], 128), (ones8, 1)]
        pred = gru(eT, hT, g1ih_m, g1hh_m, h_b, "g1")

        # pred.T in bf16 (2 chunks of (128, BL))
        predT = []
        for ki in range(2):
            ps = PSS.tile([128, BL], F32, tag="sm", name="sm")
            nc.tensor.transpose(ps[:], pred[:, ki * 128:(ki + 1) * 128],
                                eye8[:])
            t = ptile([128, BL], BF16, f"predT{ki}")
            nc.vector.tensor_copy(t[:], ps[:])
            predT.append(t)

        # u_pred.T per block: 4 chunks of (128, BL) f32 (ACT bias columns)
        up_sb = {}
        for blk in ("lo", "hi"):
            res = []
            for mi in range(4):
                ps = PSS.tile([128, BL], F32, tag="sm", name="sm")
                for j in range(2):
                    nc.tensor.matmul(
                        ps[:], upT[blk][:, j * NP_ + mi * 128:
                                        j * NP_ + mi * 128 + 128],
                        predT[j][:], start=(j == 0), stop=(j == 1))
                t = ptile([128, BL], F32, f"up{blk}{mi}")
                nc.vector.tensor_copy(t[:], ps[:])
                res.append(t)
            up_sb[blk] = res

        # ---------------- phase B: coverage attention ----------------
        def half(blk, nC, L, Lsubs, Av, ctx_f, bs):
            """Attention for batches [bs, bs+4): matmuls + tanh + e_t +
            softmax + context columns.  Two pair-groups per psum set."""
            pe = [PSS.tile([BL, l1 - l0], F32, tag="sm", name="sm")
                  for (l0, l1) in Lsubs]
            nK = (6 + 2) if blk == "lo" else (3 + 2)
            nC_ = nC
            for mi in range(4):
                ths = [TH.tile([128, L], BF16, tag="tanh", name="tanh")
                       for _ in range(4)]
                for si, (l0, l1) in enumerate(Lsubs):
                    ln = l1 - l0
                    pts = [PSB.tile([128, 512], F32, tag="pt", name="pt")
                           for _ in range(2)]
                    for kc in range(nK):
                        w = wT[blk][:, kc * NP_ + mi * 128:
                                    kc * NP_ + mi * 128 + 128]
                        for q in range(2):
                            qq = bs // 2 + q
                            if kc < nC_:
                                rhs = Av[:, kc * BL + 2 * qq:
                                         kc * BL + 2 * qq + 2, l0:l1]
                            else:
                                base = (kc - nC_) * 2 * L + 2 * l0
                                rhs = of[blk][:, base: base + 2 * ln]
                            nc.tensor.matmul(pts[q][:, 0:2 * ln], w, rhs,
                                             start=(kc == 0),
                                             stop=(kc == nK - 1))
                    for bi in range(4):
                        b = bs + bi
                        nc.scalar.activation(
                            ths[bi][:, l0:l1],
                            pts[bi // 2][:, (bi % 2) * ln:(bi % 2 + 1) * ln],
                            AF.Tanh, bias=up_sb[blk][mi][:, b:b + 1])
                for bi in range(4):
                    b = bs + bi
                    for si, (l0, l1) in enumerate(Lsubs):
                        nc.tensor.matmul(
                            pe[si][:, :],
                            nu8[blk][:, mi * B + b * BL: mi * B + (b + 1) * BL],
                            ths[bi][:, l0:l1],
                            start=(mi == 0 and bi == 0),
                            stop=(mi == 3 and bi == 3))

            # softmax over L (only rows [bs, bs+4) hold real data)
            et = ST.tile([BL, L], F32, tag="et", name="et", bufs=2)
            for si, (l0, l1) in enumerate(Lsubs):
                nc.vector.tensor_copy(et[:, l0:l1], pe[si][:, :])
            nmax = ST.tile([BL, 1], F32, tag="nmax", name="nmax")
            nc.vector.tensor_reduce(nmax[:], et[:], axis=AX.X, op=ALU.max,
                                    negate=True)
            expt = ST.tile([BL, L], BF16, tag="expt", name="expt", bufs=2)
            sums = ST.tile([BL, 1], F32, tag="sums", name="sums")
            nc.scalar.activation(expt[:], et[:], AF.Exp, bias=nmax[:],
                                 accum_out=sums[:])
            recip = ST.tile([BL, 1], F32, tag="recip", name="recip")
            nc.vector.reciprocal(recip[:], sums[:])
            alph = ST.tile([BL, L], BF16, tag="alph", name="alph", bufs=2)
            nc.vector.tensor_scalar_mul(alph[:], expt[:], recip[:])

            for bi in range(4):
                b = bs + bi
                ab = ST.tile([128, L], BF16, tag="ab", name="ab", bufs=4)
                for si, (l0, l1) in enumerate(Lsubs):
                    ln = l1 - l0
                    abp = PSS.tile([128, 512], F32, tag="sm", name="sm")
                    nc.tensor.matmul(abp[:, :ln],
                                     bsel[:, b * 128:(b + 1) * 128],
                                     alph[:, l0:l1], start=True, stop=True)
                    nc.vector.tensor_copy(ab[:, l0:l1], abp[:, :ln])
                for ci in range(nC_):
                    junk = ST.tile([128, L], BF16, tag="junk", name="junk",
                                   bufs=4)
                    nc.vector.scalar_tensor_tensor(
                        junk[:], Av[:, ci * BL + b, :], 1.0, ab[:],
                        ALU.mult, ALU.mult,
                        accum_out=ctx_f[ci][:, b:b + 1])

        def block(blk, nC, L, Lsubs, A_t):
            Av = A_t.rearrange("p (x l) -> p x l", l=L)   # (128, nC*BL, L)
            ctx_f = [ptile([128, BL], F32, f"{blk}ctx{ci}")
                     for ci in range(nC)]
            half(blk, nC, L, Lsubs, Av, ctx_f, 0)
            half(blk, nC, L, Lsubs, Av, ctx_f, 4)
            out = []
            for ci in range(nC):
                t = ptile([128, BL], BF16, f"{blk}ctxb{ci}")
                nc.vector.tensor_copy(t[:], ctx_f[ci][:])
                out.append(t)
            return out

        ctx_low = block("lo", 6, LL, [(0, LL)], A_low)
        ctx_high = block("hi", 3, LH, HSUB, A_high)
        ctxT = ctx_low + ctx_high            # 9 chunks, zero-padded rows

        # ---------------- phase C: gru2 + output ----------------
        lh_ih2 = [(t, 128) for t in ctxT] + [(ones8, 1)]
        lh_hh2 = [(predT[0], 128), (predT[1], 128), (ones8, 1)]
        new_h = gru(lh_ih2, lh_hh2, g2ih_m, g2hh_m, pred, "g2")
        nc.sync.dma_start(newh_d[:], new_h[:])

        # new_h.T bf16
        nhT = []
        for ki in range(2):
            ps = PSS.tile([128, BL], F32, tag="sm", name="sm")
            nc.tensor.transpose(ps[:], new_h[:, ki * 128:(ki + 1) * 128],
                                eye8[:])
            t = ptile([128, BL], BF16, f"nhT{ki}")
            nc.vector.tensor_copy(t[:], ps[:])
            nhT.append(t)

        # S.T = embedded.T + W_s @ new_h.T + W_c @ ctx.T   (per EMB chunk)
        sT = []
        for mi in range(2):
            ps = PSS.tile([128, BL], F32, tag="sm", name="sm")
            steps = [(WsT_m[:, j * EMB + mi * 128: j * EMB + mi * 128 + 128],
                      nhT[j]) for j in range(2)]
            steps += [(WcT_m[:, j * EMB + mi * 128: j * EMB + mi * 128 + 128],
                       ctxT[j]) for j in range(9)]
            for i, (w, r) in enumerate(steps):
                nc.tensor.matmul(ps[:], w, r[:], start=(i == 0),
                                 stop=(i == len(steps) - 1))
            t = ptile([128, BL], BF16, f"sT{mi}")
            nc.vector.tensor_tensor(t[:], ps[:], embT_f32[mi][:], ALU.add)
            sT.append(t)

        # out = S @ W_o.T : (BL, NCLS)
        ps = PSS.tile([BL, NCLS], F32, tag="sm", name="sm")
        for ki in range(2):
            nc.tensor.matmul(ps[:], sT[ki][:],
                             WoT_m[:, ki * NCLS:(ki + 1) * NCLS],
                             start=(ki == 0), stop=(ki == 1))
        out_sb = ST.tile([BL, NCLS], F32, tag="outsb", name="outsb")
        nc.vector.tensor_copy(out_sb[:], ps[:])
        nc.sync.dma_start(out_d[:], out_sb[:])

    nc.compile()
    return nc


# --------------------------------------------------------------------------
# host side
# --------------------------------------------------------------------------

def _fold128(M, dt):
    """(R, W) -> (128, ceil(R/128)*W): 128-row blocks along the free dim,
    zero padded."""
    R, W = M.shape
    nj = (R + 127) // 128
    out = np.zeros((128, nj, W), dt)
    for j in range(nj):
        kk = min(128, R - j * 128)
        out[:kk, j] = M[j * 128: j * 128 + kk]
    return out.reshape(128, nj * W)


def _fold_blocks(M, bounds, dt):
    """rows of M grouped by (start, size) bounds, each padded to 128."""
    W = M.shape[1]
    out = np.zeros((128, len(bounds), W), dt)
    for j, (s, n) in enumerate(bounds):
        out[:n, j] = M[s:s + n]
    return out.reshape(128, len(bounds) * W)


def _bsel():
    s = np.zeros((BL, BL * 128), np.float32)
    for b in range(BL):
        s[b, b * 128:(b + 1) * 128] = 1.0
    return s.astype(NPBF)


def _nu_mask(nu):
    D = np.zeros((NP_, BL, BL), np.float32)
    D[:, np.arange(BL), np.arange(BL)] = np.asarray(nu, np.float32)[:, None]
    return _fold128(D.reshape(NP_, B), NPBF)


def _pack_a(a, nC, L):
    """(BL, C, L) f32 -> (128, nC*BL*L) bf16, (j, b, l) free layout,
    zero-padded partial chunk."""
    C = a.shape[1]
    out = np.zeros((128, nC, BL, L), NPBF)
    for j in range(nC):
        kk = min(128, C - j * 128)
        out[:kk, j] = np.moveaxis(a[:, j * 128: j * 128 + kk], 0, 1)
    return out.reshape(128, nC * BL * L)


def _dup_outf(outfT, Lsubs):
    """(OF, L) out_f.T -> (128, 2*sum(2*ln)): each (o, sub) chunk
    duplicated side by side for batch-pair matmuls."""
    parts = []
    for o in range(2):
        chunk = outfT[o * 128:(o + 1) * 128]
        for (l0, l1) in Lsubs:
            c = chunk[:, l0:l1]
            parts.append(np.concatenate([c, c], axis=1))
    return np.ascontiguousarray(np.concatenate(parts, axis=1)).astype(NPBF)


def _prep_in_maps(inputs):
    f32 = np.float32

    def A(name):
        return np.asarray(inputs[name])

    x = A("x").reshape(B).astype(np.int64)
    hidden = A("hidden").astype(f32)          # (1, B, H)
    low = A("low_res").astype(f32).reshape(B, CL, LL)
    high = A("high_res").astype(f32).reshape(B, CH, LH)

    def outf(alpha, fc_w, fc_b, L):
        a = np.asarray(alpha, f32)
        o = np.asarray(fc_b, f32).reshape(L, OF)
        if np.any(a):
            o = o + (a.sum(0) @ np.asarray(fc_w, f32).T).reshape(L, OF)
        return np.ascontiguousarray(o.T)          # (OF, L) f32

    T = lambda name: np.asarray(inputs[name], f32).T

    def gru_w(w_name, b_name, bounds):
        wt = T(w_name)
        bias = A(b_name).astype(f32)[None]
        M = np.vstack([wt, bias])
        return _fold_blocks(M, bounds + [(M.shape[0] - 1, 1)], NPBF)

    rep = {
        "emb_m": _fold128(A("embedding").astype(f32), f32),
        "eye8": np.eye(BL, dtype=f32),
        "bsel": _bsel(),
        "g1ih_m": gru_w("gru1_w_ih", "gru1_b_ih", _chunks(EMB)),
        "g1hh_m": gru_w("gru1_w_hh", "gru1_b_hh", _chunks(H)),
        "g2ih_m": gru_w("gru2_w_ih", "gru2_b_ih", CTX_CH),
        "g2hh_m": gru_w("gru2_w_hh", "gru2_b_hh", _chunks(H)),
        "upT_low": _fold128(T("U_pred_low"), NPBF),
        "upT_high": _fold128(T("U_pred_high"), NPBF),
        "wT_low": np.concatenate([_fold128(T("U_a_low"), NPBF),
                                  _fold128(T("U_f_low"), NPBF)], axis=1),
        "wT_high": np.concatenate([_fold128(T("U_a_high"), NPBF),
                                   _fold128(T("U_f_high"), NPBF)], axis=1),
        "of_low": _dup_outf(outf(inputs["alpha_low"], inputs["fc_w_low"],
                                 inputs["fc_b_low"], LL), [(0, LL)]),
        "of_high": _dup_outf(outf(inputs["alpha_high"], inputs["fc_w_high"],
                                  inputs["fc_b_high"], LH), HSUB),
        "nu8_lo": _nu_mask(A("nu_low")),
        "nu8_hi": _nu_mask(A("nu_high")),
        "WsT_m": _fold128(T("W_s"), NPBF),
        "WcT_m": _fold_blocks(T("W_c"), CTX_CH, NPBF),
        "WoT_m": _fold128(T("W_o"), NPBF),
    }

    in_maps = []
    for c in range(NCORES):
        b0, b1 = c * BL, (c + 1) * BL
        sel = np.zeros((NCLS, BL), f32)
        sel[x[b0:b1], np.arange(BL)] = 1.0
        m = dict(rep)
        m["sel_m"] = _fold128(sel, f32)
        m["hT_m"] = _fold128(np.ascontiguousarray(hidden[0, b0:b1].T), NPBF)
        m["h_b"] = np.ascontiguousarray(hidden[0, b0:b1])
        m["A_low"] = _pack_a(low[b0:b1], 6, LL)
        m["A_high"] = _pack_a(high[b0:b1], 3, LH)
        in_maps.append(m)
    return in_maps


def _run(inputs, trace=False, trace_cores=None):
    if "nc" not in _STATE:
        _STATE["nc"] = _build_program()
    nc = _STATE["nc"]
    in_maps = _prep_in_maps(inputs)
    res = run_bass_kernel_spmd(nc, in_maps, list(range(NCORES)), trace=trace,
                               trace_cores=trace_cores)
    out = np.concatenate([res.results[c]["out"] for c in range(NCORES)], axis=0)
    new_h = np.concatenate([res.results[c]["new_h"] for c in range(NCORES)],
                           axis=0)[None]
    return (out.astype(np.float32), new_h.astype(np.float32)), res


def kernel(**inputs):
    (out, new_h), _ = _run(inputs)
    return out, new_h
